# revision 29
# baseline (speedup 1.0000x reference)
"""CapsNet dynamic-routing kernel for Trainium2 (8 NeuronCores, SPMD).

Math (see reference):
  u_hat[j,b,k,u] = sum_d W[j,k,d,u] * x[b,k,d]
  for j in 0..9:  (sequential, b_IJ carried)
    3 routing iterations:
      c_k      = softmax(b_IJ, axis=1)[:, j]
      s[b,u]   = sum_k c_k u_hat[j,b,k,u]
      v        = squash(s)
      agree[k] = sum_{b,u} u_hat[j,b,k,u] v[b,u]   (sum over FULL batch)
      b_IJ[:, j] += agree
  out[b,j,u] = v (last iteration of each j)

Distribution: data-parallel over batch (64 per core).  The only cross-core
quantity is agree -> per routing iteration each core contributes its batch
partial [1152]; an AllGather (floor ~4.6us vs AllReduce ~9.7us) collects the
8 partials and a 3-op DVE tree reduces them locally.

v2 changes vs the AllReduce baseline:
  - single k-major layout (p = khat*8 + d) for x, W, and all softmax state;
    the d-major duals (x_b / wj_all) are gone.  The AllGather return DMA
    replicates the [16,72] agree rows to all 128 partitions via stride-0
    source dims, so no partition-broadcast copies are needed.
  - W band placement (block-diag wbd tiles for u_hat) is done with 16
    on-chip tensor_copies per capsule from a resident dense k-major W image
    instead of per-capsule scatter DMAs.  The old DMA path emitted ~18k
    16-byte packets per capsule on the shared DMA engines, which starved
    the AllReduce return path by ~6us per routing iteration.
  - agree matmuls run third-major so each PSUM third is drained to the
    k-major bounce vector while the PE streams the next third.
  - junk fillers cut from 72-76 wide matmuls per window to a handful: the
    wide fillers burned enough power across 8 cores to duty-cycle-throttle
    the PE (378us at k=4/n=8 in the baseline profile), halving the real
    s/agree matmul rate.  u_hat of the next capsule provides most of the
    keep-warm work in the shorter AllGather window.

All matmul operands are bf16 (fp32 matmuls double-pass on the PE);
accumulation is fp32 in PSUM, and all routing state stays fp32.
(j=0,t=0) softmax is skipped: b_IJ=0 there so c=1/J exactly; the s matvec
uses a host-prescaled W_0/J tile.
Outputs accumulate in SBUF (v_all) and ship in one final DMA.
"""

import numpy as np
import ml_dtypes

import concourse.bass as bass
import concourse.bacc as bacc
import concourse.mybir as mybir
import concourse.tile as tile
from concourse.tile import add_dep_helper
from concourse import bass_utils

F32 = mybir.dt.float32
BF16 = mybir.dt.bfloat16
AF = mybir.ActivationFunctionType
ALU = mybir.AluOpType

J = 10        # output capsules
K = 1152      # input capsules
D = 8         # in dim
U = 16        # out dim
B = 512       # batch
N_CORES = 8
ITERS = 3
EPS = 1e-7
G = K // 16   # 72 groups of 16 k

# wide junk fillers per AR window (t -> count); small: they only bridge the
# gap between the next capsule's u_hat work and the s matvec.
WIDE_FILL = {0: 6, 1: 6, 2: 6}
WIDE_FILL_LAST = 22   # j = J-1 windows have no next-capsule u_hat work
SQUASH_FILL = 0       # squash-gap fillers delayed agree 1:1 when half-clocked


def capsnet_body(tc, nc, x_dram, wb_dram, w0p_dram, sel_dram, out_dram,
                 replica_groups, b_local):
    """Emit the per-core program. x [128,G,b] bf16, wb [128, J*G*16] bf16,
    w0p [128, G*16] bf16, out [b, J, U] fp32."""
    from contextlib import ExitStack
    es = ExitStack()
    p_const = es.enter_context(tc.tile_pool(name="const", bufs=1))
    p_uhat = es.enter_context(tc.tile_pool(name="uhat", bufs=2))
    p_cw = es.enter_context(tc.tile_pool(name="cw", bufs=2))
    p_small = es.enter_context(tc.tile_pool(name="small", bufs=4))
    p_v = es.enter_context(tc.tile_pool(name="vpool", bufs=2))
    p_agr = es.enter_context(tc.tile_pool(name="agr", bufs=2))
    p_ps_uh = es.enter_context(tc.tile_pool(name="ps_uh", bufs=3, space="PSUM"))
    p_ps_s = es.enter_context(tc.tile_pool(name="ps_s", bufs=1, space="PSUM"))
    p_ps_a = es.enter_context(tc.tile_pool(name="ps_a", bufs=1, space="PSUM"))
    p_dram = es.enter_context(tc.tile_pool(name="dram", bufs=4, space="DRAM"))

    # ---- persistent tiles (all k-major: p = khat*8 + d)
    x_a = p_const.tile([128, G, b_local], BF16, tag="x_a")
    wbd_a = p_const.tile([128, G, 256], BF16, tag="wbd_a")
    wbd_b = p_const.tile([128, G, 256], BF16, tag="wbd_b")
    wbsrc = p_const.tile([128, J, G, U], BF16, tag="wbsrc")
    wj0p = p_const.tile([128, G, U], BF16, tag="wj0p")
    v_all = p_const.tile([b_local, J, U], F32, tag="v_all")
    # softmax state lives on the 16 khat partitions only; the final c is
    # broadcast to all 128 partitions by a selector matmul on the PE.
    e3 = p_const.tile([16, G, J], F32, tag="e3")
    den = p_const.tile([16, G], F32, tag="den")  # sum_j e3, kept incrementally
    sel = p_const.tile([128, 128], BF16, tag="sel")
    ctmp = p_const.tile([128, G], BF16, tag="ctmp")
    wbds = [wbd_a, wbd_b]

    # ---- startup loads: wbsrc capsules 0-1 first (gate the j0/j1 band
    # placements), x on gpsimd, rest of wbsrc behind.
    CH = G * U
    nc.scalar.dma_start(wj0p[:], w0p_dram.ap()[:])
    nc.scalar.dma_start(sel[:], sel_dram.ap()[:])
    wsv = wbsrc.rearrange("p j g u -> p (j g u)")
    nc.scalar.dma_start(wsv[:, 0:2 * CH], wb_dram.ap()[:, 0:2 * CH])
    for cidx in range(4):
        sl = slice(cidx * 18, (cidx + 1) * 18)
        nc.gpsimd.dma_start(x_a[:, sl, :], x_dram.ap()[:, sl])
    for j0 in range(2, J, 2):
        nc.scalar.dma_start(
            wsv[:, j0 * CH:(j0 + 2) * CH], wb_dram.ap()[:, j0 * CH:(j0 + 2) * CH]
        )

    # zeros for the block-diag tiles; bands only touch 1/16 of the cols so
    # the zero background is written once and never again.
    for h in range(2):
        nc.vector.memset(wbd_a[:, 36 * h:36 * (h + 1), :], 0.0)
    nc.vector.memset(e3[:], 1.0)
    nc.vector.memset(den[:], float(J))
    # rows 16..127 must stay 0 so the selector matmul contraction only
    # picks up the 16 live khat rows
    nc.vector.memset(ctmp[:], 0.0)

    A_tiles = {}

    def emit_band_place(j, rs=range(16)):
        """Place capsule j's block-diag W bands into wbds[j%2]: h-split
        3-dim DMAs from the host-packed contiguous wb_dram [128,(j g h u)].
        Engine tensor-copies can't do it (partition starts must be
        32-aligned; bands sit at 8-partition offsets).  All transfers ride
        the gpsimd SWDGE queue: its descriptor generator aggregates the
        16-byte runs into multi-partition packets, so the shared hardware
        DMA engines stay clean for the AllGather bounce path."""
        wbd = wbds[j % 2]
        dstv = wbd.rearrange("p g (h kk u) -> p g h kk u", h=2, kk=16)
        for r in rs:
            for h in range(2):
                src = bass.AP(
                    wb_dram, (8 * r) * (J * CH) + j * CH + h * 8,
                    [[J * CH, 8], [16, G], [1, 8]],
                )
                nc.gpsimd.dma_start(dstv[8 * r:8 * r + 8, :, h, r, :], src)

    def emit_uhat_mms(j, g_lo, g_hi, after=None):
        """PE matmuls + PSUM->SBUF copies for groups [g_lo, g_hi) of capsule j.
        `after`: ordering-only dep so the scheduler runs these in the
        AllGather window (after the agree matmuls), not earlier."""
        if j not in A_tiles:
            # partitions (h, b): even-u u_hat on 0..63, odd-u on 64..127
            A_tiles[j] = p_uhat.tile(
                [128, U // 2, G, 16], BF16, tag="uhat", name=f"uhat{j}"
            )
        A = A_tiles[j]
        wbd = wbds[j % 2]
        last_mm = None
        for gq in range(g_lo // 4, g_hi // 4):
            ps = p_ps_uh.tile([128, 512], F32, tag="ps_uh", name="ps_uh")
            for i in range(4):
                g = gq * 4 + i
                for h in range(2):
                    last_mm = nc.tensor.matmul(
                        ps[h * 64:(h + 1) * 64, i * 128:(i + 1) * 128],
                        x_a[:, g, :], wbd[:, g, h * 128:(h + 1) * 128],
                        start=True, stop=True,
                    )
                    if after is not None:
                        add_dep_helper(last_mm.ins, after.ins, sync=False,
                                       reason="uhat after agree")
            # all copies on DVE: a Copy activation on ACT would evict the
            # preloaded Sqrt/Exp tables and put a reload on the critical path.
            dst = A[:, :, gq * 4:gq * 4 + 4, :].transpose((0, 2, 1, 3))
            src_v = ps.rearrange("p (g k u) -> p g k u", k=16, u=U // 2)
            src_v = src_v.transpose((0, 1, 3, 2))
            nc.vector.tensor_copy(dst, src_v)
        return last_mm

    p_ps_f = es.enter_context(tc.tile_pool(name="ps_f", bufs=1, space="PSUM"))

    def emit_filler(j, n, after=None):
        """Independent wide matmuls with no consumers: keep the PE warm
        while real work is blocked (results are discarded).  N=512.  Reads
        x_a/wbsrc only -- touching wbd here would make the band-placement
        copies wait on filler drains."""
        wf = wbsrc.rearrange("p j g u -> p (j g u)")
        fps = p_ps_f.tile([b_local, 512], F32, tag="fps", name="fps")
        first_mm = last_mm = None
        for f in range(n):
            fs = f % 22
            last_mm = nc.tensor.matmul(
                fps[:], x_a[:, f % G, :], wf[:, fs * 512:(fs + 1) * 512],
                start=True, stop=True,
            )
            if first_mm is None:
                first_mm = last_mm
            if after is not None:
                add_dep_helper(last_mm.ins, after.ins, sync=False,
                               reason="filler ordering")
        return first_mm, last_mm

    pe_tail = None  # ordering anchor: last PE inst of the prev AG window

    for j in range(J):
        A = None
        for t in range(ITERS):
            last = (j == J - 1) and (t == ITERS - 1)
            first = (j == 0) and (t == 0)
            if first:
                # b_IJ = 0 => c = 1/J exactly; use the host-prescaled W_0/J
                cw = wj0p
            else:
                # softmax column j: c = e3[:,:,j] / den (den kept incrementally)
                rec = p_small.tile([16, G], F32, tag="rec")
                nc.vector.reciprocal(rec[:], den[:])
                # ctmp rows 16..127 are zeroed once at startup; the selector
                # matmul c128[m, g] = sum_p S[p, m] ctmp[p, g] = ctmp[m//8, g]
                # broadcasts c to all 128 k-major partitions on the (idle)
                # PE.  Stride-0 DMA partition broadcasts with a non-outer
                # broadcast dim read garbage on HW, hence this route.
                nc.vector.tensor_mul(ctmp[0:16, :], e3[:, :, j], rec[:])
                c_ps = p_ps_s.tile([128, G], F32, tag="c_ps", name="c_ps")
                cmm = nc.tensor.matmul(c_ps[:], sel[:], ctmp[:],
                                       start=True, stop=True)
                if pe_tail is not None:
                    add_dep_helper(cmm.ins, pe_tail.ins, sync=False,
                                   reason="c bcast after AG-window fillers")
                # cW = W_j * c (c broadcast over u, read from PSUM); two
                # halves so the s matvec can start while the second half is
                # still computing
                cw = p_cw.tile([128, G, U], BF16, tag="cw")
                GH = G // 2
                for h in range(2):
                    sl = slice(h * GH, (h + 1) * GH)
                    nc.vector.tensor_mul(
                        cw[:, sl, :], wbsrc[:, j, sl, :],
                        c_ps[:, sl].unsqueeze(2).broadcast_to((128, GH, U)),
                    )
            # s matvec: accumulate over groups
            s_ps = p_ps_s.tile([b_local, U], F32, tag="s_ps")
            for g in range(G):
                mm = nc.tensor.matmul(
                    s_ps[:], x_a[:, g, :], cw[:, g, :],
                    start=(g == 0), stop=(g == G - 1),
                )
                if g == 0 and pe_tail is not None:
                    add_dep_helper(mm.ins, pe_tail.ins, sync=False,
                                   reason="s after AG-window fillers")
            if not last:
                emit_filler(j, SQUASH_FILL, after=mm)
            # squash: v = s * ssq / ((1+ssq)(sqrt(ssq)+EPS))
            # ssq via DVE mult+reduce (keeps ACT on the Sqrt table)
            s_sb = p_small.tile([b_local, U], F32, tag="s_sb")
            shadow = p_small.tile([b_local, U], F32, tag="shadow")
            ssq = p_small.tile([b_local, 1], F32, tag="ssq")
            sq1 = p_small.tile([b_local, 1], F32, tag="sq1")
            sqr = p_small.tile([b_local, 1], F32, tag="sqr")
            dn2 = p_small.tile([b_local, 1], F32, tag="dn2")
            rc2 = p_small.tile([b_local, 1], F32, tag="rc2")
            fac = p_small.tile([b_local, 1], F32, tag="fac")
            nc.vector.tensor_copy(s_sb[:], s_ps[:])
            nc.vector.tensor_mul(shadow[:], s_sb[:], s_sb[:])
            nc.vector.tensor_reduce(ssq[:], shadow[:], mybir.AxisListType.X, ALU.add)
            nc.scalar.sqrt(sqr[:], ssq[:])
            nc.vector.tensor_scalar_add(sq1[:], ssq[:], 1.0)
            nc.vector.scalar_tensor_tensor(
                dn2[:], sqr[:], EPS, sq1[:], ALU.add, ALU.mult
            )
            nc.vector.reciprocal(rc2[:], dn2[:])
            nc.vector.tensor_mul(fac[:], ssq[:], rc2[:])
            if not last:
                # preload the Exp ACT table during the AllGather window
                # (anchored on fac so it runs after this squash)
                dxp = p_small.tile([b_local, 1], F32, tag="dxp")
                nc.scalar.activation(dxp[:], fac[:], AF.Exp)
            if first:
                # u_hat(0) band placement + matmuls run on the PE while the
                # (0,0) squash proceeds on DVE: the s matvec above only
                # needed x_a + the prescaled W_0/J tile
                emit_band_place(0)
                emit_uhat_mms(0, 0, G)
                for h2 in range(2):
                    nc.vector.memset(wbd_b[:, 36 * h2:36 * (h2 + 1), :], 0.0)
                emit_band_place(1)
            if t == ITERS - 1:
                # v cols are parity-ordered (h,uhat); un-permute into the
                # SBUF output accumulator (single strided DVE op)
                dstv = v_all[:, j, :].rearrange("b (u h) -> b u h", h=2)
                srcv = s_ps.rearrange("b (h u) -> b h u", h=2)
                srcv = srcv.transpose((0, 2, 1))
                nc.vector.tensor_scalar_mul(dstv, srcv, fac[:])
            if last:
                pe_tail = None
                break
            # agree matvec, third-major: each PSUM third is drained to the
            # k-major bounce vector while the PE streams the next third.
            if A is None:
                A = A_tiles.pop(j)
            v_bf = p_v.tile([b_local, U], BF16, tag="v_bf")
            nc.vector.tensor_scalar_mul(v_bf[:], s_ps[:], fac[:])
            # v2[(h,b), q] = v[b, 2q+h]: contract over 128 partitions.
            v2 = p_v.tile([128, U // 2], BF16, tag="v2")
            nc.vector.tensor_copy(v2[0:64, :], v_bf[:, 0:8])
            nc.vector.tensor_copy(v2[64:128, :], v_bf[:, 8:16])
            # two 384-col slots (1 PSUM bank each); third 2 reuses slot 0
            # after its drain, which completes during third 1's compute
            aps3 = p_ps_a.tile([1, 1024], F32, tag="ps_a3", name="ps_a3")
            agr_sb = p_agr.tile([1, K], BF16, tag="agr_sb")
            sb_v = agr_sb.rearrange("p (k c g) -> p k c g", k=16, c=3)
            agree_last = None
            for third in range(3):
                off = (third % 2) * 512
                for q in range(U // 2):
                    agree_last = nc.tensor.matmul(
                        aps3[:, off:off + 384],
                        v2[:, q:q + 1],
                        A[:, q, third * 24:(third + 1) * 24, :],
                        start=(q == 0), stop=(q == U // 2 - 1),
                    )
                src_v = aps3[:, off:off + 384]
                src_v = src_v.rearrange("p (g k) -> p k g", k=16)
                nc.vector.tensor_copy(sb_v[:, :, third, :], src_v)
            fill_j = j + 1 if j + 1 < J else j
            # collective bounce: agr_sb is khat-major so the replicated
            # return DMA has 288-byte contiguous runs.
            cc_in = p_dram.tile([1, K], BF16, tag="cc_in")
            cc_out = nc.dram_tensor(
                f"ccout_{j}_{t}", [N_CORES, K], BF16, addr_space="Shared"
            )
            nc.sync.dma_start(cc_in[:], agr_sb[:])
            cc_inst = nc.gpsimd.collective_compute(
                "AllGather", ALU.bypass,
                replica_groups=replica_groups,
                ins=[cc_in[:].opt()], outs=[cc_out.ap().opt()],
            )
            # PE schedule for the AG window: a short tail filler covers the
            # bounce copy/DMA, then the next capsule's u_hat, then a few
            # junk fillers; the PE then blocks at the next s matvec
            # (ordering dep via pe_tail).
            _, tail = emit_filler(fill_j, 6, after=agree_last)
            if j + 1 < J:
                tail = emit_uhat_mms(j + 1, t * 24, (t + 1) * 24, after=tail)
                wide_n = WIDE_FILL[t]
            else:
                wide_n = WIDE_FILL_LAST
            _, pe_tail = emit_filler(fill_j, wide_n, after=tail)
            if j + 2 < J:
                # place j+2's W bands into wbds[j%2] (last reader u_hat(j)
                # finished during capsule j-1, so these never block).
                # Spread over the 3 windows; they run in the AG-window
                # slack on gpsimd (post-trigger) and vector.
                rs = (range(0, 6), range(6, 11), range(11, 16))[t]
                emit_band_place(j + 2, rs)
            # AllGather return: one 3-dim DMA onto the 16 khat partitions
            # [16, rank, G]; a 3-op DVE tree reduces the 8 rank partials.
            agr8 = p_agr.tile([16, N_CORES, G], BF16, tag="agr8")
            t4 = p_agr.tile([16, 2, G], F32, tag="t4")
            gsum = p_agr.tile([16, G], F32, tag="gsum")
            agr = p_agr.tile([16, G], F32, tag="agr")
            eag = p_agr.tile([16, G], F32, tag="eag")
            # two queues halve the return transfer's serial latency
            src_lo = bass.AP(cc_out, 0, [[G, 16], [K, 4], [1, G]])
            src_hi = bass.AP(cc_out, 4 * K, [[G, 16], [K, 4], [1, G]])
            nc.sync.dma_start(agr8[:, 0:4, :], src_lo)
            nc.scalar.dma_start(agr8[:, 4:8, :], src_hi)
            # rank-partials reduce on vector; fp32 accumulation from the
            # bf16 wire payloads
            nc.vector.tensor_add(t4[:], agr8[:, 0:2, :], agr8[:, 2:4, :])
            nc.vector.tensor_add(gsum[:], agr8[:, 4, :], agr8[:, 5, :])
            nc.vector.tensor_add(t4[:, 0, :], t4[:, 0, :], t4[:, 1, :])
            nc.vector.tensor_add(gsum[:], gsum[:], agr8[:, 6, :])
            nc.vector.tensor_add(gsum[:], gsum[:], agr8[:, 7, :])
            nc.vector.tensor_add(agr[:], t4[:, 0, :], gsum[:])
            nc.scalar.activation(eag[:], agr[:], AF.Exp)
            # preload Sqrt table for the next squash (anchored on eag)
            dsq = p_small.tile([16, 1], F32, tag="dsq")
            nc.scalar.activation(dsq[:], eag[0:16, 0:1], AF.Sqrt)
            # delta = (eag-1)*e3_j keeps den incremental; then update e3
            delta = p_small.tile([16, G], F32, tag="delta")
            nc.vector.scalar_tensor_tensor(
                delta[:], eag[:], -1.0, e3[:, :, j], ALU.add, ALU.mult
            )
            nc.vector.tensor_mul(e3[:, :, j], e3[:, :, j], eag[:])
            nc.vector.tensor_add(den[:], den[:], delta[:])

    # single output DMA at the end
    nc.sync.dma_start(out_dram.ap()[:], v_all[:])
    es.close()


def build_nc(n_cores=N_CORES, b_local=B // N_CORES):
    nc = bacc.Bacc(
        "TRN2", target_bir_lowering=False, debug=False,
        num_devices=n_cores,
    )
    x_dram = nc.dram_tensor("x_k", [128, G, b_local], BF16, kind="ExternalInput")
    wb_dram = nc.dram_tensor("w_bands", [128, J * G * U], BF16, kind="ExternalInput")
    w0p_dram = nc.dram_tensor("w_j0p", [128, G * U], BF16, kind="ExternalInput")
    sel_dram = nc.dram_tensor("sel", [128, 128], BF16, kind="ExternalInput")
    out_dram = nc.dram_tensor("out", [b_local, J, U], F32, kind="ExternalOutput")
    rg = [list(range(n_cores))]
    with tile.TileContext(nc) as tc:
        capsnet_body(tc, nc, x_dram, wb_dram, w0p_dram, sel_dram, out_dram,
                     rg, b_local)
    nc.compile()
    return nc


def shard_x(x_full):
    """x_full [B,1152,8,1] -> per-core [128, G, b] bf16 k-major
    (p = khat*8 + d)."""
    b_local = x_full.shape[0] // N_CORES
    shards = []
    for i in range(N_CORES):
        xs = np.ascontiguousarray(
            x_full[i * b_local:(i + 1) * b_local, :, :, 0], dtype=np.float32
        )
        r = xs.reshape(b_local, G, 16, D)
        x_a = r.transpose(2, 3, 1, 0).reshape(128, G, b_local)  # k-major
        shards.append(np.ascontiguousarray(x_a.astype(ml_dtypes.bfloat16)))
    return shards


_NC_CACHE = {}


def prep_w(W):
    """W [J,K,D,U] fp32 -> host-packed staging images (all bf16):
      wb  [128, J*G*16]  k-major  wb[khat*8+d, j, g, (h,uhat)]
      w0p [128, G*16]    wb[:, 0] / J  (uniform-softmax shortcut for j=0,t=0)
    where w_par[j,k,d,h,uhat] = W[j,k,d,2*uhat+h] (parity-split u)."""
    w = np.asarray(W, dtype=np.float32).reshape(J, K, D, 8, 2)
    w_par = w.transpose(0, 1, 2, 4, 3).reshape(J, G, 16, D, U)
    # wb: [khat, d, j, g, u]
    wb = w_par.transpose(2, 3, 0, 1, 4).reshape(128, J, G, U)
    w0p = wb[:, 0] * (1.0 / J)
    to = lambda a: np.ascontiguousarray(
        a.reshape(128, -1).astype(ml_dtypes.bfloat16))
    return to(wb), to(w0p)


def make_sel():
    """S[p, m] = 1 iff p == m//8: the selector matmul S^T @ ctmp broadcasts
    16-partition khat rows to all 128 k-major partitions."""
    S = np.zeros((128, 128), np.float32)
    for m in range(128):
        S[m // 8, m] = 1.0
    return np.ascontiguousarray(S.astype(ml_dtypes.bfloat16))


def kernel(inputs, W, num_outputs):
    assert int(num_outputs) == J
    x_full = np.asarray(inputs, dtype=np.float32)
    wb, w0p = prep_w(W)
    assert x_full.shape == (B, K, D, 1)

    if "nc" not in _NC_CACHE:
        _NC_CACHE["nc"] = build_nc()
    nc = _NC_CACHE["nc"]

    shards = shard_x(x_full)
    sel = make_sel()
    in_maps = [
        {"x_k": shards[i], "w_bands": wb, "w_j0p": w0p, "sel": sel}
        for i in range(N_CORES)
    ]
    res = bass_utils.run_bass_kernel_spmd(
        nc, in_maps, core_ids=list(range(N_CORES))
    )
    outs = [res.results[i]["out"] for i in range(N_CORES)]  # [b, J, U] each
    full = np.concatenate(outs, axis=0)  # [B, J, U]
    return full[..., None].astype(np.float32)


# revision 30
# speedup vs baseline: 1.0167x; 1.0167x over previous
"""CapsNet dynamic-routing kernel for Trainium2 (8 NeuronCores, SPMD).

Math (see reference):
  u_hat[j,b,k,u] = sum_d W[j,k,d,u] * x[b,k,d]
  for j in 0..9:  (sequential, b_IJ carried)
    3 routing iterations:
      c_k      = softmax(b_IJ, axis=1)[:, j]
      s[b,u]   = sum_k c_k u_hat[j,b,k,u]
      v        = squash(s)
      agree[k] = sum_{b,u} u_hat[j,b,k,u] v[b,u]   (sum over FULL batch)
      b_IJ[:, j] += agree
  out[b,j,u] = v (last iteration of each j)

Distribution: data-parallel over batch (64 per core).  The only cross-core
quantity is agree: per routing iteration each core ships its bf16 batch
partial [1152] through an AllGather (mesh floor ~4.6us vs AllReduce ~9.7us
at 8 cores) and a DVE tree sums the 8 partials locally in fp32.

Layout: single k-major partition order p = khat*8 + d for x, W, and all
routing state (v1 kept d-major duals of x and W just to make partition
broadcasts block-copyable).  The softmax state (e3, den) lives on the 16
khat partitions; c is broadcast to all 128 partitions by a selector matmul
c128 = S^T @ ctmp on the otherwise-idle PE (S[p,m] = 1 iff p == m//8, a
host input).  Stride-0-DMA partition broadcasts with a non-outermost
broadcast dim read garbage on HW, and engine copies cannot start at
non-32-aligned partitions, so the PE is the only clean path.

W band placement (block-diag wbd tiles for u_hat) is per-capsule 3-dim
scatter DMAs from the host-packed contiguous wb image, all routed through
the gpsimd SWDGE queue: its descriptor generator aggregates the 16-byte
runs into multi-partition packets.  v1 put 1/3 of these on the scalar
HWDGE queue, whose ~6k tiny packets per window starved the shared DMA
engines and delayed the collective return path by ~6us per iteration.

agree runs third-major into two bank-aligned PSUM slots (matmul
accumulation regions must not straddle the 2KB bank boundary); each third
drains to the khat-major bounce vector while the PE streams the next.

The PE duty-cycle governor (HAM, k=4/n=8 half-clock telemetry) tracks
recent PE activity density: junk fillers bridge the AllGather window so
the s/agree matmuls that follow are not half-clocked.  Oversized fillers
delay the softmax chain 1:1 (queue is in-order), so they are tuned small;
u_hat of the next capsule provides most of the keep-warm work.  fp8e4 for
the u_hat/agree path was tried and is NOT worth it: plain float8e4 does
not double-pump the PE (only the packed _x4 formats do) and the extra x/W
images cost startup time (measured 1074us vs 1052us, rel err 1.4e-2).

All matmul operands are bf16 (fp32 matmuls double-pass on the PE);
accumulation is fp32 in PSUM, and all routing state stays fp32.
(j=0,t=0) softmax is skipped: b_IJ=0 there so c=1/J exactly; the s matvec
uses a host-prescaled W_0/J tile, and runs before the u_hat(0) matmuls so
the (0,0) squash overlaps them.
Outputs accumulate in SBUF (v_all) and ship in one final DMA.

Measured on HW: 1052us (v1 AllReduce baseline: 1175us), rel err 3.1e-3.
"""

import numpy as np
import ml_dtypes

import concourse.bass as bass
import concourse.bacc as bacc
import concourse.mybir as mybir
import concourse.tile as tile
from concourse.tile import add_dep_helper
from concourse import bass_utils

F32 = mybir.dt.float32
BF16 = mybir.dt.bfloat16
AF = mybir.ActivationFunctionType
ALU = mybir.AluOpType

J = 10        # output capsules
K = 1152      # input capsules
D = 8         # in dim
U = 16        # out dim
B = 512       # batch
N_CORES = 8
ITERS = 3
EPS = 1e-7
G = K // 16   # 72 groups of 16 k

# wide junk fillers per AR window (t -> count); small: they only bridge the
# gap between the next capsule's u_hat work and the s matvec.
WIDE_FILL = {0: 10, 1: 10, 2: 10}
WIDE_FILL_LAST = 26   # j = J-1 windows have no next-capsule u_hat work
SQUASH_FILL = 0       # squash-gap fillers delayed agree 1:1 when half-clocked


def capsnet_body(tc, nc, x_dram, wb_dram, w0p_dram, sel_dram, out_dram,
                 replica_groups, b_local):
    """Emit the per-core program. x [128,G,b] bf16, wb [128, J*G*16] bf16,
    w0p [128, G*16] bf16, out [b, J, U] fp32."""
    from contextlib import ExitStack
    es = ExitStack()
    p_const = es.enter_context(tc.tile_pool(name="const", bufs=1))
    p_uhat = es.enter_context(tc.tile_pool(name="uhat", bufs=2))
    p_cw = es.enter_context(tc.tile_pool(name="cw", bufs=2))
    p_small = es.enter_context(tc.tile_pool(name="small", bufs=4))
    p_v = es.enter_context(tc.tile_pool(name="vpool", bufs=2))
    p_agr = es.enter_context(tc.tile_pool(name="agr", bufs=2))
    p_ps_uh = es.enter_context(tc.tile_pool(name="ps_uh", bufs=3, space="PSUM"))
    p_ps_s = es.enter_context(tc.tile_pool(name="ps_s", bufs=1, space="PSUM"))
    p_ps_a = es.enter_context(tc.tile_pool(name="ps_a", bufs=1, space="PSUM"))
    p_dram = es.enter_context(tc.tile_pool(name="dram", bufs=4, space="DRAM"))

    # ---- persistent tiles (all k-major: p = khat*8 + d)
    x_a = p_const.tile([128, G, b_local], BF16, tag="x_a")
    wbd_a = p_const.tile([128, G, 256], BF16, tag="wbd_a")
    wbd_b = p_const.tile([128, G, 256], BF16, tag="wbd_b")
    wbsrc = p_const.tile([128, J, G, U], BF16, tag="wbsrc")
    wj0p = p_const.tile([128, G, U], BF16, tag="wj0p")
    v_all = p_const.tile([b_local, J, U], F32, tag="v_all")
    # softmax state lives on the 16 khat partitions only; the final c is
    # broadcast to all 128 partitions by a selector matmul on the PE.
    e3 = p_const.tile([16, G, J], F32, tag="e3")
    den = p_const.tile([16, G], F32, tag="den")  # sum_j e3, kept incrementally
    sel = p_const.tile([128, 128], BF16, tag="sel")
    ctmp = p_const.tile([128, G], BF16, tag="ctmp")
    wbds = [wbd_a, wbd_b]

    # ---- startup loads: wbsrc capsules 0-1 first (gate the j0/j1 band
    # placements), x on gpsimd, rest of wbsrc behind.
    CH = G * U
    nc.scalar.dma_start(wj0p[:], w0p_dram.ap()[:])
    nc.scalar.dma_start(sel[:], sel_dram.ap()[:])
    wsv = wbsrc.rearrange("p j g u -> p (j g u)")
    nc.scalar.dma_start(wsv[:, 0:2 * CH], wb_dram.ap()[:, 0:2 * CH])
    for cidx in range(4):
        sl = slice(cidx * 18, (cidx + 1) * 18)
        nc.gpsimd.dma_start(x_a[:, sl, :], x_dram.ap()[:, sl])
    for j0 in range(2, J, 2):
        nc.scalar.dma_start(
            wsv[:, j0 * CH:(j0 + 2) * CH], wb_dram.ap()[:, j0 * CH:(j0 + 2) * CH]
        )

    # zeros for the block-diag tiles; bands only touch 1/16 of the cols so
    # the zero background is written once and never again.
    for h in range(2):
        nc.vector.memset(wbd_a[:, 36 * h:36 * (h + 1), :], 0.0)
    nc.vector.memset(e3[:], 1.0)
    nc.vector.memset(den[:], float(J))
    # rows 16..127 must stay 0 so the selector matmul contraction only
    # picks up the 16 live khat rows
    nc.vector.memset(ctmp[:], 0.0)

    A_tiles = {}

    def emit_band_place(j, rs=range(16)):
        """Place capsule j's block-diag W bands into wbds[j%2]: h-split
        3-dim DMAs from the host-packed contiguous wb_dram [128,(j g h u)].
        Engine tensor-copies can't do it (partition starts must be
        32-aligned; bands sit at 8-partition offsets).  All transfers ride
        the gpsimd SWDGE queue: its descriptor generator aggregates the
        16-byte runs into multi-partition packets, so the shared hardware
        DMA engines stay clean for the AllGather bounce path."""
        wbd = wbds[j % 2]
        dstv = wbd.rearrange("p g (h kk u) -> p g h kk u", h=2, kk=16)
        for r in rs:
            for h in range(2):
                src = bass.AP(
                    wb_dram, (8 * r) * (J * CH) + j * CH + h * 8,
                    [[J * CH, 8], [16, G], [1, 8]],
                )
                nc.gpsimd.dma_start(dstv[8 * r:8 * r + 8, :, h, r, :], src)

    def emit_uhat_mms(j, g_lo, g_hi, after=None):
        """PE matmuls + PSUM->SBUF copies for groups [g_lo, g_hi) of capsule j.
        `after`: ordering-only dep so the scheduler runs these in the
        AllGather window (after the agree matmuls), not earlier."""
        if j not in A_tiles:
            # partitions (h, b): even-u u_hat on 0..63, odd-u on 64..127
            A_tiles[j] = p_uhat.tile(
                [128, U // 2, G, 16], BF16, tag="uhat", name=f"uhat{j}"
            )
        A = A_tiles[j]
        wbd = wbds[j % 2]
        last_mm = None
        for gq in range(g_lo // 4, g_hi // 4):
            ps = p_ps_uh.tile([128, 512], F32, tag="ps_uh", name="ps_uh")
            for i in range(4):
                g = gq * 4 + i
                for h in range(2):
                    last_mm = nc.tensor.matmul(
                        ps[h * 64:(h + 1) * 64, i * 128:(i + 1) * 128],
                        x_a[:, g, :], wbd[:, g, h * 128:(h + 1) * 128],
                        start=True, stop=True,
                    )
                    if after is not None:
                        add_dep_helper(last_mm.ins, after.ins, sync=False,
                                       reason="uhat after agree")
            # all copies on DVE: a Copy activation on ACT would evict the
            # preloaded Sqrt/Exp tables and put a reload on the critical path.
            dst = A[:, :, gq * 4:gq * 4 + 4, :].transpose((0, 2, 1, 3))
            src_v = ps.rearrange("p (g k u) -> p g k u", k=16, u=U // 2)
            src_v = src_v.transpose((0, 1, 3, 2))
            nc.vector.tensor_copy(dst, src_v)
        return last_mm

    p_ps_f = es.enter_context(tc.tile_pool(name="ps_f", bufs=1, space="PSUM"))

    def emit_filler(j, n, after=None):
        """Independent wide matmuls with no consumers: keep the PE warm
        while real work is blocked (results are discarded).  N=512.  Reads
        x_a/wbsrc only -- touching wbd here would make the band-placement
        copies wait on filler drains."""
        wf = wbsrc.rearrange("p j g u -> p (j g u)")
        fps = p_ps_f.tile([b_local, 512], F32, tag="fps", name="fps")
        first_mm = last_mm = None
        for f in range(n):
            fs = f % 22
            last_mm = nc.tensor.matmul(
                fps[:], x_a[:, f % G, :], wf[:, fs * 512:(fs + 1) * 512],
                start=True, stop=True,
            )
            if first_mm is None:
                first_mm = last_mm
            if after is not None:
                add_dep_helper(last_mm.ins, after.ins, sync=False,
                               reason="filler ordering")
        return first_mm, last_mm

    pe_tail = None  # ordering anchor: last PE inst of the prev AG window

    for j in range(J):
        A = None
        for t in range(ITERS):
            last = (j == J - 1) and (t == ITERS - 1)
            first = (j == 0) and (t == 0)
            if first:
                # b_IJ = 0 => c = 1/J exactly; use the host-prescaled W_0/J
                cw = wj0p
            else:
                # softmax column j: c = e3[:,:,j] / den (den kept incrementally)
                rec = p_small.tile([16, G], F32, tag="rec")
                nc.vector.reciprocal(rec[:], den[:])
                # ctmp rows 16..127 are zeroed once at startup; the selector
                # matmul c128[m, g] = sum_p S[p, m] ctmp[p, g] = ctmp[m//8, g]
                # broadcasts c to all 128 k-major partitions on the (idle)
                # PE.  Stride-0 DMA partition broadcasts with a non-outer
                # broadcast dim read garbage on HW, hence this route.
                nc.vector.tensor_mul(ctmp[0:16, :], e3[:, :, j], rec[:])
                c_ps = p_ps_s.tile([128, G], F32, tag="c_ps", name="c_ps")
                cmm = nc.tensor.matmul(c_ps[:], sel[:], ctmp[:],
                                       start=True, stop=True)
                if pe_tail is not None:
                    add_dep_helper(cmm.ins, pe_tail.ins, sync=False,
                                   reason="c bcast after AG-window fillers")
                # cW = W_j * c (c broadcast over u, read from PSUM); two
                # halves so the s matvec can start while the second half is
                # still computing
                cw = p_cw.tile([128, G, U], BF16, tag="cw")
                GH = G // 2
                for h in range(2):
                    sl = slice(h * GH, (h + 1) * GH)
                    nc.vector.tensor_mul(
                        cw[:, sl, :], wbsrc[:, j, sl, :],
                        c_ps[:, sl].unsqueeze(2).broadcast_to((128, GH, U)),
                    )
            # s matvec: accumulate over groups
            s_ps = p_ps_s.tile([b_local, U], F32, tag="s_ps")
            for g in range(G):
                mm = nc.tensor.matmul(
                    s_ps[:], x_a[:, g, :], cw[:, g, :],
                    start=(g == 0), stop=(g == G - 1),
                )
                if g == 0 and pe_tail is not None:
                    add_dep_helper(mm.ins, pe_tail.ins, sync=False,
                                   reason="s after AG-window fillers")
            if not last:
                emit_filler(j, SQUASH_FILL, after=mm)
            # squash: v = s * ssq / ((1+ssq)(sqrt(ssq)+EPS))
            # ssq via DVE mult+reduce (keeps ACT on the Sqrt table)
            s_sb = p_small.tile([b_local, U], F32, tag="s_sb")
            shadow = p_small.tile([b_local, U], F32, tag="shadow")
            ssq = p_small.tile([b_local, 1], F32, tag="ssq")
            sq1 = p_small.tile([b_local, 1], F32, tag="sq1")
            sqr = p_small.tile([b_local, 1], F32, tag="sqr")
            dn2 = p_small.tile([b_local, 1], F32, tag="dn2")
            rc2 = p_small.tile([b_local, 1], F32, tag="rc2")
            fac = p_small.tile([b_local, 1], F32, tag="fac")
            nc.vector.tensor_copy(s_sb[:], s_ps[:])
            nc.vector.tensor_mul(shadow[:], s_sb[:], s_sb[:])
            nc.vector.tensor_reduce(ssq[:], shadow[:], mybir.AxisListType.X, ALU.add)
            nc.scalar.sqrt(sqr[:], ssq[:])
            nc.vector.tensor_scalar_add(sq1[:], ssq[:], 1.0)
            nc.vector.scalar_tensor_tensor(
                dn2[:], sqr[:], EPS, sq1[:], ALU.add, ALU.mult
            )
            nc.vector.reciprocal(rc2[:], dn2[:])
            nc.vector.tensor_mul(fac[:], ssq[:], rc2[:])
            if not last:
                # preload the Exp ACT table during the AllGather window
                # (anchored on fac so it runs after this squash)
                dxp = p_small.tile([b_local, 1], F32, tag="dxp")
                nc.scalar.activation(dxp[:], fac[:], AF.Exp)
            if first:
                # u_hat(0) band placement + matmuls run on the PE while the
                # (0,0) squash proceeds on DVE: the s matvec above only
                # needed x_a + the prescaled W_0/J tile
                emit_band_place(0)
                emit_uhat_mms(0, 0, G)
                for h2 in range(2):
                    nc.vector.memset(wbd_b[:, 36 * h2:36 * (h2 + 1), :], 0.0)
                emit_band_place(1)
            if t == ITERS - 1:
                # v cols are parity-ordered (h,uhat); un-permute into the
                # SBUF output accumulator (single strided DVE op)
                dstv = v_all[:, j, :].rearrange("b (u h) -> b u h", h=2)
                srcv = s_ps.rearrange("b (h u) -> b h u", h=2)
                srcv = srcv.transpose((0, 2, 1))
                nc.vector.tensor_scalar_mul(dstv, srcv, fac[:])
            if last:
                pe_tail = None
                break
            # agree matvec, third-major: each PSUM third is drained to the
            # k-major bounce vector while the PE streams the next third.
            if A is None:
                A = A_tiles.pop(j)
            v_bf = p_v.tile([b_local, U], BF16, tag="v_bf")
            nc.vector.tensor_scalar_mul(v_bf[:], s_ps[:], fac[:])
            # v2[(h,b), q] = v[b, 2q+h]: contract over 128 partitions.
            v2 = p_v.tile([128, U // 2], BF16, tag="v2")
            nc.vector.tensor_copy(v2[0:64, :], v_bf[:, 0:8])
            nc.vector.tensor_copy(v2[64:128, :], v_bf[:, 8:16])
            # two 384-col slots (1 PSUM bank each); third 2 reuses slot 0
            # after its drain, which completes during third 1's compute
            aps3 = p_ps_a.tile([1, 1024], F32, tag="ps_a3", name="ps_a3")
            agr_sb = p_agr.tile([1, K], BF16, tag="agr_sb")
            sb_v = agr_sb.rearrange("p (k c g) -> p k c g", k=16, c=3)
            agree_last = None
            for third in range(3):
                off = (third % 2) * 512
                for q in range(U // 2):
                    agree_last = nc.tensor.matmul(
                        aps3[:, off:off + 384],
                        v2[:, q:q + 1],
                        A[:, q, third * 24:(third + 1) * 24, :],
                        start=(q == 0), stop=(q == U // 2 - 1),
                    )
                src_v = aps3[:, off:off + 384]
                src_v = src_v.rearrange("p (g k) -> p k g", k=16)
                nc.vector.tensor_copy(sb_v[:, :, third, :], src_v)
            fill_j = j + 1 if j + 1 < J else j
            # collective bounce: agr_sb is khat-major so the replicated
            # return DMA has 288-byte contiguous runs.
            cc_in = p_dram.tile([1, K], BF16, tag="cc_in")
            cc_out = nc.dram_tensor(
                f"ccout_{j}_{t}", [N_CORES, K], BF16, addr_space="Shared"
            )
            nc.sync.dma_start(cc_in[:], agr_sb[:])
            cc_inst = nc.gpsimd.collective_compute(
                "AllGather", ALU.bypass,
                replica_groups=replica_groups,
                ins=[cc_in[:].opt()], outs=[cc_out.ap().opt()],
            )
            # PE schedule for the AG window: a short tail filler covers the
            # bounce copy/DMA, then the next capsule's u_hat, then a few
            # junk fillers; the PE then blocks at the next s matvec
            # (ordering dep via pe_tail).
            _, tail = emit_filler(fill_j, 6, after=agree_last)
            if j + 1 < J:
                tail = emit_uhat_mms(j + 1, t * 24, (t + 1) * 24, after=tail)
                wide_n = WIDE_FILL[t]
            else:
                wide_n = WIDE_FILL_LAST
            _, pe_tail = emit_filler(fill_j, wide_n, after=tail)
            if j + 2 < J:
                # place j+2's W bands into wbds[j%2] (last reader u_hat(j)
                # finished during capsule j-1, so these never block).
                # Spread over the 3 windows; they run in the AG-window
                # slack on gpsimd (post-trigger) and vector.
                rs = (range(0, 6), range(6, 11), range(11, 16))[t]
                emit_band_place(j + 2, rs)
            # AllGather return: one 3-dim DMA onto the 16 khat partitions
            # [16, rank, G]; a 3-op DVE tree reduces the 8 rank partials.
            agr8 = p_agr.tile([16, N_CORES, G], BF16, tag="agr8")
            t4 = p_agr.tile([16, 2, G], F32, tag="t4")
            gsum = p_agr.tile([16, G], F32, tag="gsum")
            agr = p_agr.tile([16, G], F32, tag="agr")
            eag = p_agr.tile([16, G], F32, tag="eag")
            # two queues halve the return transfer's serial latency
            src_lo = bass.AP(cc_out, 0, [[G, 16], [K, 4], [1, G]])
            src_hi = bass.AP(cc_out, 4 * K, [[G, 16], [K, 4], [1, G]])
            nc.sync.dma_start(agr8[:, 0:4, :], src_lo)
            nc.scalar.dma_start(agr8[:, 4:8, :], src_hi)
            # rank-partials reduce on vector; fp32 accumulation from the
            # bf16 wire payloads
            nc.vector.tensor_add(t4[:], agr8[:, 0:2, :], agr8[:, 2:4, :])
            nc.vector.tensor_add(gsum[:], agr8[:, 4, :], agr8[:, 5, :])
            nc.vector.tensor_add(t4[:, 0, :], t4[:, 0, :], t4[:, 1, :])
            nc.vector.tensor_add(gsum[:], gsum[:], agr8[:, 6, :])
            nc.vector.tensor_add(gsum[:], gsum[:], agr8[:, 7, :])
            nc.vector.tensor_add(agr[:], t4[:, 0, :], gsum[:])
            nc.scalar.activation(eag[:], agr[:], AF.Exp)
            # preload Sqrt table for the next squash (anchored on eag)
            dsq = p_small.tile([16, 1], F32, tag="dsq")
            nc.scalar.activation(dsq[:], eag[0:16, 0:1], AF.Sqrt)
            # delta = (eag-1)*e3_j keeps den incremental; then update e3
            delta = p_small.tile([16, G], F32, tag="delta")
            nc.vector.scalar_tensor_tensor(
                delta[:], eag[:], -1.0, e3[:, :, j], ALU.add, ALU.mult
            )
            nc.vector.tensor_mul(e3[:, :, j], e3[:, :, j], eag[:])
            nc.vector.tensor_add(den[:], den[:], delta[:])

    # single output DMA at the end
    nc.sync.dma_start(out_dram.ap()[:], v_all[:])
    es.close()


def build_nc(n_cores=N_CORES, b_local=B // N_CORES):
    nc = bacc.Bacc(
        "TRN2", target_bir_lowering=False, debug=False,
        num_devices=n_cores,
    )
    x_dram = nc.dram_tensor("x_k", [128, G, b_local], BF16, kind="ExternalInput")
    wb_dram = nc.dram_tensor("w_bands", [128, J * G * U], BF16, kind="ExternalInput")
    w0p_dram = nc.dram_tensor("w_j0p", [128, G * U], BF16, kind="ExternalInput")
    sel_dram = nc.dram_tensor("sel", [128, 128], BF16, kind="ExternalInput")
    out_dram = nc.dram_tensor("out", [b_local, J, U], F32, kind="ExternalOutput")
    rg = [list(range(n_cores))]
    with tile.TileContext(nc) as tc:
        capsnet_body(tc, nc, x_dram, wb_dram, w0p_dram, sel_dram, out_dram,
                     rg, b_local)
    nc.compile()
    return nc


def shard_x(x_full):
    """x_full [B,1152,8,1] -> per-core [128, G, b] bf16 k-major
    (p = khat*8 + d)."""
    b_local = x_full.shape[0] // N_CORES
    shards = []
    for i in range(N_CORES):
        xs = np.ascontiguousarray(
            x_full[i * b_local:(i + 1) * b_local, :, :, 0], dtype=np.float32
        )
        r = xs.reshape(b_local, G, 16, D)
        x_a = r.transpose(2, 3, 1, 0).reshape(128, G, b_local)  # k-major
        shards.append(np.ascontiguousarray(x_a.astype(ml_dtypes.bfloat16)))
    return shards


_NC_CACHE = {}


def prep_w(W):
    """W [J,K,D,U] fp32 -> host-packed staging images (all bf16):
      wb  [128, J*G*16]  k-major  wb[khat*8+d, j, g, (h,uhat)]
      w0p [128, G*16]    wb[:, 0] / J  (uniform-softmax shortcut for j=0,t=0)
    where w_par[j,k,d,h,uhat] = W[j,k,d,2*uhat+h] (parity-split u)."""
    w = np.asarray(W, dtype=np.float32).reshape(J, K, D, 8, 2)
    w_par = w.transpose(0, 1, 2, 4, 3).reshape(J, G, 16, D, U)
    # wb: [khat, d, j, g, u]
    wb = w_par.transpose(2, 3, 0, 1, 4).reshape(128, J, G, U)
    w0p = wb[:, 0] * (1.0 / J)
    to = lambda a: np.ascontiguousarray(
        a.reshape(128, -1).astype(ml_dtypes.bfloat16))
    return to(wb), to(w0p)


def make_sel():
    """S[p, m] = 1 iff p == m//8: the selector matmul S^T @ ctmp broadcasts
    16-partition khat rows to all 128 k-major partitions."""
    S = np.zeros((128, 128), np.float32)
    for m in range(128):
        S[m // 8, m] = 1.0
    return np.ascontiguousarray(S.astype(ml_dtypes.bfloat16))


def kernel(inputs, W, num_outputs):
    assert int(num_outputs) == J
    x_full = np.asarray(inputs, dtype=np.float32)
    wb, w0p = prep_w(W)
    assert x_full.shape == (B, K, D, 1)

    if "nc" not in _NC_CACHE:
        _NC_CACHE["nc"] = build_nc()
    nc = _NC_CACHE["nc"]

    shards = shard_x(x_full)
    sel = make_sel()
    in_maps = [
        {"x_k": shards[i], "w_bands": wb, "w_j0p": w0p, "sel": sel}
        for i in range(N_CORES)
    ]
    res = bass_utils.run_bass_kernel_spmd(
        nc, in_maps, core_ids=list(range(N_CORES))
    )
    outs = [res.results[i]["out"] for i in range(N_CORES)]  # [b, J, U] each
    full = np.concatenate(outs, axis=0)  # [B, J, U]
    return full[..., None].astype(np.float32)


# revision 32
# speedup vs baseline: 1.0514x; 1.0341x over previous
"""CapsNet dynamic-routing kernel for Trainium2 (8 NeuronCores, SPMD).

Math (see reference):
  u_hat[j,b,k,u] = sum_d W[j,k,d,u] * x[b,k,d]
  for j in 0..9:  (sequential, b_IJ carried)
    3 routing iterations:
      c_k      = softmax(b_IJ, axis=1)[:, j]
      s[b,u]   = sum_k c_k u_hat[j,b,k,u]
      v        = squash(s)
      agree[k] = sum_{b,u} u_hat[j,b,k,u] v[b,u]   (sum over FULL batch)
      b_IJ[:, j] += agree
  out[b,j,u] = v (last iteration of each j)

Distribution: data-parallel over batch (64 per core).  The only cross-core
quantity is agree: per routing iteration each core ships its bf16 batch
partial [1152] through an AllGather (mesh floor ~4.6us vs AllReduce ~9.7us
at 8 cores) and a DVE tree sums the 8 partials locally in fp32.

Layout: single k-major partition order p = khat*8 + d for x, W, and all
routing state (v1 kept d-major duals of x and W just to make partition
broadcasts block-copyable).  The softmax state (e3, den) lives on the 16
khat partitions; c is broadcast to all 128 partitions by a selector matmul
c128 = S^T @ ctmp on the otherwise-idle PE (S[p,m] = 1 iff p == m//8, a
host input).  Stride-0-DMA partition broadcasts with a non-outermost
broadcast dim read garbage on HW, and engine copies cannot start at
non-32-aligned partitions, so the PE is the only clean path.

W band placement (block-diag wbd tiles for u_hat) is per-capsule 3-dim
scatter DMAs from the host-packed contiguous wb image, all routed through
the gpsimd SWDGE queue: its descriptor generator aggregates the 16-byte
runs into multi-partition packets.  v1 put 1/3 of these on the scalar
HWDGE queue, whose ~6k tiny packets per window starved the shared DMA
engines and delayed the collective return path by ~6us per iteration.

agree runs third-major into two bank-aligned PSUM slots (matmul
accumulation regions must not straddle the 2KB bank boundary); each third
drains to the khat-major bounce vector while the PE streams the next.

The PE duty-cycle governor (HAM, k=4/n=8 half-clock telemetry) tracks
recent PE activity density: junk fillers bridge the AllGather window so
the s/agree matmuls that follow are not half-clocked.  Oversized fillers
delay the softmax chain 1:1 (queue is in-order), so they are tuned small;
u_hat of the next capsule provides most of the keep-warm work.  fp8e4 for
the u_hat/agree path was tried and is NOT worth it: plain float8e4 does
not double-pump the PE (only the packed _x4 formats do) and the extra x/W
images cost startup time (measured 1074us vs 1052us, rel err 1.4e-2).

All matmul operands are bf16 (fp32 matmuls double-pass on the PE);
accumulation is fp32 in PSUM, and all routing state stays fp32.
(j=0,t=0) softmax is skipped: b_IJ=0 there so c=1/J exactly; the s matvec
uses a host-prescaled W_0/J tile, and runs before the u_hat(0) matmuls so
the (0,0) squash overlaps them.
Outputs accumulate in SBUF (v_all) and ship in one final DMA.

Measured on HW: 1052us (v1 AllReduce baseline: 1175us), rel err 3.1e-3.
"""

import numpy as np
import ml_dtypes

import concourse.bass as bass
import concourse.bacc as bacc
import concourse.mybir as mybir
import concourse.tile as tile
from concourse.tile import add_dep_helper
from concourse import bass_utils

F32 = mybir.dt.float32
BF16 = mybir.dt.bfloat16
AF = mybir.ActivationFunctionType
ALU = mybir.AluOpType

J = 10        # output capsules
K = 1152      # input capsules
D = 8         # in dim
U = 16        # out dim
B = 512       # batch
N_CORES = 8
ITERS = 3
EPS = 1e-7
G = K // 16   # 72 groups of 16 k

# wide junk fillers per AR window (t -> count); small: they only bridge the
# gap between the next capsule's u_hat work and the s matvec.
WIDE_FILL = {0: 10, 1: 10, 2: 10}
WIDE_FILL_LAST = 26   # j = J-1 windows have no next-capsule u_hat work
SQUASH_FILL = 0       # squash-gap fillers delayed agree 1:1 when half-clocked


def capsnet_body(tc, nc, x_dram, wb_dram, w0p_dram, sel_dram, out_dram,
                 replica_groups, b_local):
    """Emit the per-core program. x [128,G,b] bf16, wb [128, J*G*16] bf16,
    w0p [128, G*16] bf16, out [b, J, U] fp32."""
    from contextlib import ExitStack
    es = ExitStack()
    p_const = es.enter_context(tc.tile_pool(name="const", bufs=1))
    p_uhat = es.enter_context(tc.tile_pool(name="uhat", bufs=2))
    p_cw = es.enter_context(tc.tile_pool(name="cw", bufs=2))
    p_small = es.enter_context(tc.tile_pool(name="small", bufs=4))
    p_v = es.enter_context(tc.tile_pool(name="vpool", bufs=2))
    p_agr = es.enter_context(tc.tile_pool(name="agr", bufs=2))
    p_ps_uh = es.enter_context(tc.tile_pool(name="ps_uh", bufs=3, space="PSUM"))
    p_ps_s = es.enter_context(tc.tile_pool(name="ps_s", bufs=1, space="PSUM"))
    p_ps_a = es.enter_context(tc.tile_pool(name="ps_a", bufs=1, space="PSUM"))
    p_dram = es.enter_context(tc.tile_pool(name="dram", bufs=4, space="DRAM"))

    # ---- persistent tiles (all k-major: p = khat*8 + d)
    x_a = p_const.tile([128, G, b_local], BF16, tag="x_a")
    wbd_a = p_const.tile([128, G, 256], BF16, tag="wbd_a")
    wbd_b = p_const.tile([128, G, 256], BF16, tag="wbd_b")
    wbsrc = p_const.tile([128, J, G, U], BF16, tag="wbsrc")
    wj0p = p_const.tile([128, G, U], BF16, tag="wj0p")
    v_all = p_const.tile([b_local, J, U], F32, tag="v_all")
    # softmax state lives on the 16 khat partitions only; the final c is
    # broadcast to all 128 partitions by a selector matmul on the PE.
    e3 = p_const.tile([16, G, J], F32, tag="e3")
    den = p_const.tile([16, G], F32, tag="den")  # sum_j e3, kept incrementally
    sel = p_const.tile([128, 128], BF16, tag="sel")
    ctmp = p_const.tile([128, G], BF16, tag="ctmp")
    wbds = [wbd_a, wbd_b]

    # ---- startup loads: wbsrc capsules 0-1 first (gate the j0/j1 band
    # placements), x on gpsimd, rest of wbsrc behind.
    CH = G * U
    nc.scalar.dma_start(wj0p[:], w0p_dram.ap()[:])
    nc.scalar.dma_start(sel[:], sel_dram.ap()[:])
    wsv = wbsrc.rearrange("p j g u -> p (j g u)")
    nc.scalar.dma_start(wsv[:, 0:2 * CH], wb_dram.ap()[:, 0:2 * CH])
    for cidx in range(4):
        sl = slice(cidx * 18, (cidx + 1) * 18)
        nc.gpsimd.dma_start(x_a[:, sl, :], x_dram.ap()[:, sl])
    for j0 in range(2, J, 2):
        nc.scalar.dma_start(
            wsv[:, j0 * CH:(j0 + 2) * CH], wb_dram.ap()[:, j0 * CH:(j0 + 2) * CH]
        )

    # zeros for the block-diag tiles; bands only touch 1/16 of the cols so
    # the zero background is written once and never again.
    for h in range(2):
        nc.vector.memset(wbd_a[:, 36 * h:36 * (h + 1), :], 0.0)
    nc.vector.memset(e3[:], 1.0)
    nc.vector.memset(den[:], float(J))
    # rows 16..127 must stay 0 so the selector matmul contraction only
    # picks up the 16 live khat rows
    nc.vector.memset(ctmp[:], 0.0)

    A_tiles = {}

    def emit_band_place(j, rs=range(16)):
        """Place capsule j's block-diag W bands into wbds[j%2]: h-split
        3-dim DMAs from the host-packed contiguous wb_dram [128,(j g h u)].
        Engine tensor-copies can't do it (partition starts must be
        32-aligned; bands sit at 8-partition offsets).  All transfers ride
        the gpsimd SWDGE queue: its descriptor generator aggregates the
        16-byte runs into multi-partition packets, so the shared hardware
        DMA engines stay clean for the AllGather bounce path."""
        wbd = wbds[j % 2]
        dstv = wbd.rearrange("p g (h kk u) -> p g h kk u", h=2, kk=16)
        for r in rs:
            for h in range(2):
                src = bass.AP(
                    wb_dram, (8 * r) * (J * CH) + j * CH + h * 8,
                    [[J * CH, 8], [16, G], [1, 8]],
                )
                nc.gpsimd.dma_start(dstv[8 * r:8 * r + 8, :, h, r, :], src)

    def emit_uhat_mms(j, g_lo, g_hi, after=None):
        """PE matmuls + PSUM->SBUF copies for groups [g_lo, g_hi) of capsule j.
        `after`: ordering-only dep so the scheduler runs these in the
        AllGather window (after the agree matmuls), not earlier."""
        if j not in A_tiles:
            # partitions (h, b): even-u u_hat on 0..63, odd-u on 64..127
            A_tiles[j] = p_uhat.tile(
                [128, U // 2, G, 16], BF16, tag="uhat", name=f"uhat{j}"
            )
        A = A_tiles[j]
        wbd = wbds[j % 2]
        last_mm = None
        for gq in range(g_lo // 4, g_hi // 4):
            ps = p_ps_uh.tile([128, 512], F32, tag="ps_uh", name="ps_uh")
            for i in range(4):
                g = gq * 4 + i
                for h in range(2):
                    last_mm = nc.tensor.matmul(
                        ps[h * 64:(h + 1) * 64, i * 128:(i + 1) * 128],
                        x_a[:, g, :], wbd[:, g, h * 128:(h + 1) * 128],
                        start=True, stop=True,
                    )
                    if after is not None:
                        add_dep_helper(last_mm.ins, after.ins, sync=False,
                                       reason="uhat after agree")
            # all copies on DVE: a Copy activation on ACT would evict the
            # preloaded Sqrt/Exp tables and put a reload on the critical path.
            dst = A[:, :, gq * 4:gq * 4 + 4, :].transpose((0, 2, 1, 3))
            src_v = ps.rearrange("p (g k u) -> p g k u", k=16, u=U // 2)
            src_v = src_v.transpose((0, 1, 3, 2))
            nc.vector.tensor_copy(dst, src_v)
        return last_mm

    p_ps_f = es.enter_context(tc.tile_pool(name="ps_f", bufs=1, space="PSUM"))

    for _f in range(30):
        fwarm = p_ps_f.tile([b_local, 512], F32, tag="fps", name="fwarm")
        nc.tensor.matmul(fwarm[:, 0:128], sel[:, 0:b_local], sel[:],
                         start=True, stop=True)

    def emit_filler(j, n, after=None):
        """Independent wide matmuls with no consumers: keep the PE warm
        while real work is blocked (results are discarded).  N=512.  Reads
        x_a/wbsrc only -- touching wbd here would make the band-placement
        copies wait on filler drains."""
        wf = wbsrc.rearrange("p j g u -> p (j g u)")
        fps = p_ps_f.tile([b_local, 512], F32, tag="fps", name="fps")
        first_mm = last_mm = None
        for f in range(n):
            fs = f % 22
            last_mm = nc.tensor.matmul(
                fps[:], x_a[:, f % G, :], wf[:, fs * 512:(fs + 1) * 512],
                start=True, stop=True,
            )
            if first_mm is None:
                first_mm = last_mm
            if after is not None:
                add_dep_helper(last_mm.ins, after.ins, sync=False,
                               reason="filler ordering")
        return first_mm, last_mm

    pe_tail = None  # ordering anchor: last PE inst of the prev AG window

    for j in range(J):
        A = None
        for t in range(ITERS):
            last = (j == J - 1) and (t == ITERS - 1)
            first = (j == 0) and (t == 0)
            if first:
                # b_IJ = 0 => c = 1/J exactly; use the host-prescaled W_0/J
                cw = wj0p
            else:
                # softmax column j: c = e3[:,:,j] / den (den kept incrementally)
                rec = p_small.tile([16, G], F32, tag="rec")
                nc.vector.reciprocal(rec[:], den[:])
                # ctmp rows 16..127 are zeroed once at startup; the selector
                # matmul c128[m, g] = sum_p S[p, m] ctmp[p, g] = ctmp[m//8, g]
                # broadcasts c to all 128 k-major partitions on the (idle)
                # PE.  Stride-0 DMA partition broadcasts with a non-outer
                # broadcast dim read garbage on HW, hence this route.
                nc.vector.tensor_mul(ctmp[0:16, :], e3[:, :, j], rec[:])
                c_ps = p_ps_s.tile([128, G], F32, tag="c_ps", name="c_ps")
                cmm = nc.tensor.matmul(c_ps[:], sel[:], ctmp[:],
                                       start=True, stop=True)
                if pe_tail is not None:
                    add_dep_helper(cmm.ins, pe_tail.ins, sync=False,
                                   reason="c bcast after AG-window fillers")
                # cW = W_j * c (c broadcast over u, read from PSUM); two
                # halves so the s matvec can start while the second half is
                # still computing
                cw = p_cw.tile([128, G, U], BF16, tag="cw")
                GH = G // 2
                for h in range(2):
                    sl = slice(h * GH, (h + 1) * GH)
                    nc.vector.tensor_mul(
                        cw[:, sl, :], wbsrc[:, j, sl, :],
                        c_ps[:, sl].unsqueeze(2).broadcast_to((128, GH, U)),
                    )
            # s matvec: accumulate over groups
            s_ps = p_ps_s.tile([b_local, U], F32, tag="s_ps")
            for g in range(G):
                mm = nc.tensor.matmul(
                    s_ps[:], x_a[:, g, :], cw[:, g, :],
                    start=(g == 0), stop=(g == G - 1),
                )
                if g == 0 and pe_tail is not None:
                    add_dep_helper(mm.ins, pe_tail.ins, sync=False,
                                   reason="s after AG-window fillers")
            if not last:
                emit_filler(j, SQUASH_FILL, after=mm)
            # squash: v = s * ssq / ((1+ssq)(sqrt(ssq)+EPS))
            # ssq via DVE mult+reduce (keeps ACT on the Sqrt table)
            s_sb = p_small.tile([b_local, U], F32, tag="s_sb")
            shadow = p_small.tile([b_local, U], F32, tag="shadow")
            ssq = p_small.tile([b_local, 1], F32, tag="ssq")
            sq1 = p_small.tile([b_local, 1], F32, tag="sq1")
            sqr = p_small.tile([b_local, 1], F32, tag="sqr")
            dn2 = p_small.tile([b_local, 1], F32, tag="dn2")
            rc2 = p_small.tile([b_local, 1], F32, tag="rc2")
            fac = p_small.tile([b_local, 1], F32, tag="fac")
            nc.vector.tensor_copy(s_sb[:], s_ps[:])
            nc.vector.tensor_mul(shadow[:], s_sb[:], s_sb[:])
            nc.vector.tensor_reduce(ssq[:], shadow[:], mybir.AxisListType.X, ALU.add)
            nc.scalar.sqrt(sqr[:], ssq[:])
            nc.vector.tensor_scalar_add(sq1[:], ssq[:], 1.0)
            nc.vector.scalar_tensor_tensor(
                dn2[:], sqr[:], EPS, sq1[:], ALU.add, ALU.mult
            )
            nc.vector.reciprocal(rc2[:], dn2[:])
            nc.vector.tensor_mul(fac[:], ssq[:], rc2[:])
            if not last:
                # preload the Exp ACT table during the AllGather window
                # (anchored on fac so it runs after this squash)
                dxp = p_small.tile([b_local, 1], F32, tag="dxp")
                nc.scalar.activation(dxp[:], fac[:], AF.Exp)
            if first:
                # u_hat(0) band placement + matmuls run on the PE while the
                # (0,0) squash proceeds on DVE: the s matvec above only
                # needed x_a + the prescaled W_0/J tile
                emit_band_place(0)
                emit_uhat_mms(0, 0, G)
                for h2 in range(2):
                    nc.vector.memset(wbd_b[:, 36 * h2:36 * (h2 + 1), :], 0.0)
                emit_band_place(1)
            if t == ITERS - 1:
                # v cols are parity-ordered (h,uhat); un-permute into the
                # SBUF output accumulator (single strided DVE op)
                dstv = v_all[:, j, :].rearrange("b (u h) -> b u h", h=2)
                srcv = s_ps.rearrange("b (h u) -> b h u", h=2)
                srcv = srcv.transpose((0, 2, 1))
                nc.vector.tensor_scalar_mul(dstv, srcv, fac[:])
            if last:
                pe_tail = None
                break
            # agree matvec, third-major: each PSUM third is drained to the
            # k-major bounce vector while the PE streams the next third.
            if A is None:
                A = A_tiles.pop(j)
            v_bf = p_v.tile([b_local, U], BF16, tag="v_bf")
            nc.vector.tensor_scalar_mul(v_bf[:], s_ps[:], fac[:])
            # v2[(h,b), q] = v[b, 2q+h]: contract over 128 partitions.
            v2 = p_v.tile([128, U // 2], BF16, tag="v2")
            nc.vector.tensor_copy(v2[0:64, :], v_bf[:, 0:8])
            nc.vector.tensor_copy(v2[64:128, :], v_bf[:, 8:16])
            # two 384-col slots (1 PSUM bank each); third 2 reuses slot 0
            # after its drain, which completes during third 1's compute
            aps3 = p_ps_a.tile([1, 1024], F32, tag="ps_a3", name="ps_a3")
            agr_sb = p_agr.tile([1, K], BF16, tag="agr_sb")
            sb_v = agr_sb.rearrange("p (k c g) -> p k c g", k=16, c=3)
            agree_last = None
            for third in range(3):
                off = (third % 2) * 512
                for q in range(U // 2):
                    agree_last = nc.tensor.matmul(
                        aps3[:, off:off + 384],
                        v2[:, q:q + 1],
                        A[:, q, third * 24:(third + 1) * 24, :],
                        start=(q == 0), stop=(q == U // 2 - 1),
                    )
                src_v = aps3[:, off:off + 384]
                src_v = src_v.rearrange("p (g k) -> p k g", k=16)
                nc.vector.tensor_copy(sb_v[:, :, third, :], src_v)
            fill_j = j + 1 if j + 1 < J else j
            # collective bounce: agr_sb is khat-major so the replicated
            # return DMA has 288-byte contiguous runs.
            cc_in = p_dram.tile([1, K], BF16, tag="cc_in")
            cc_out = nc.dram_tensor(
                f"ccout_{j}_{t}", [N_CORES, K], BF16, addr_space="Shared"
            )
            nc.sync.dma_start(cc_in[:], agr_sb[:])
            cc_inst = nc.gpsimd.collective_compute(
                "AllGather", ALU.bypass,
                replica_groups=replica_groups,
                ins=[cc_in[:].opt()], outs=[cc_out.ap().opt()],
            )
            # PE schedule for the AG window: a short tail filler covers the
            # bounce copy/DMA, then the next capsule's u_hat, then a few
            # junk fillers; the PE then blocks at the next s matvec
            # (ordering dep via pe_tail).
            _, tail = emit_filler(fill_j, 6, after=agree_last)
            if j + 1 < J:
                tail = emit_uhat_mms(j + 1, t * 24, (t + 1) * 24, after=tail)
                wide_n = WIDE_FILL[t]
            else:
                wide_n = WIDE_FILL_LAST
            _, pe_tail = emit_filler(fill_j, wide_n, after=tail)
            # AllGather return: one 3-dim DMA onto the 16 khat partitions
            # [16, rank, G]; a 3-op DVE tree reduces the 8 rank partials.
            agr8 = p_agr.tile([16, N_CORES, G], BF16, tag="agr8")
            t8 = p_agr.tile([16, 4, G], F32, tag="t8")
            t4 = p_agr.tile([16, 2, G], F32, tag="t4")
            agr = p_agr.tile([16, G], F32, tag="agr")
            eag = p_agr.tile([16, G], F32, tag="eag")
            # four queues quarter the return transfer's serial latency; the
            # gpsimd quarter is its first instruction after the AG wait
            for qi, eng in enumerate((nc.sync, nc.scalar, nc.sync,
                                      nc.gpsimd)):
                src_q = bass.AP(cc_out, qi * 2 * K, [[G, 16], [K, 2], [1, G]])
                eng.dma_start(agr8[:, qi * 2:(qi + 1) * 2, :], src_q)
            # rank-partials reduce on vector: 3-op tree, fp32 accumulation
            # from the bf16 wire payloads
            nc.vector.tensor_add(t8[:], agr8[:, 0:4, :], agr8[:, 4:8, :])
            nc.vector.tensor_add(t4[:], t8[:, 0:2, :], t8[:, 2:4, :])
            nc.vector.tensor_add(agr[:], t4[:, 0, :], t4[:, 1, :])
            if j + 2 < J:
                # place j+2's W bands into wbds[j%2] (last reader u_hat(j)
                # finished during capsule j-1, so these never block); the
                # transfers process in the post-AG chain's shadow
                rs = (range(0, 6), range(6, 11), range(11, 16))[t]
                emit_band_place(j + 2, rs)
            nc.scalar.activation(eag[:], agr[:], AF.Exp)
            # preload Sqrt table for the next squash (anchored on eag)
            dsq = p_small.tile([16, 1], F32, tag="dsq")
            nc.scalar.activation(dsq[:], eag[0:16, 0:1], AF.Sqrt)
            # delta = (eag-1)*e3_j keeps den incremental; then update e3
            delta = p_small.tile([16, G], F32, tag="delta")
            nc.vector.scalar_tensor_tensor(
                delta[:], eag[:], -1.0, e3[:, :, j], ALU.add, ALU.mult
            )
            nc.vector.tensor_mul(e3[:, :, j], e3[:, :, j], eag[:])
            nc.vector.tensor_add(den[:], den[:], delta[:])

    # single output DMA at the end
    nc.sync.dma_start(out_dram.ap()[:], v_all[:])
    es.close()


def build_nc(n_cores=N_CORES, b_local=B // N_CORES):
    nc = bacc.Bacc(
        "TRN2", target_bir_lowering=False, debug=False,
        num_devices=n_cores,
    )
    x_dram = nc.dram_tensor("x_k", [128, G, b_local], BF16, kind="ExternalInput")
    wb_dram = nc.dram_tensor("w_bands", [128, J * G * U], BF16, kind="ExternalInput")
    w0p_dram = nc.dram_tensor("w_j0p", [128, G * U], BF16, kind="ExternalInput")
    sel_dram = nc.dram_tensor("sel", [128, 128], BF16, kind="ExternalInput")
    out_dram = nc.dram_tensor("out", [b_local, J, U], F32, kind="ExternalOutput")
    rg = [list(range(n_cores))]
    with tile.TileContext(nc) as tc:
        capsnet_body(tc, nc, x_dram, wb_dram, w0p_dram, sel_dram, out_dram,
                     rg, b_local)
    nc.compile()
    return nc


def shard_x(x_full):
    """x_full [B,1152,8,1] -> per-core [128, G, b] bf16 k-major
    (p = khat*8 + d)."""
    b_local = x_full.shape[0] // N_CORES
    shards = []
    for i in range(N_CORES):
        xs = np.ascontiguousarray(
            x_full[i * b_local:(i + 1) * b_local, :, :, 0], dtype=np.float32
        )
        r = xs.reshape(b_local, G, 16, D)
        x_a = r.transpose(2, 3, 1, 0).reshape(128, G, b_local)  # k-major
        shards.append(np.ascontiguousarray(x_a.astype(ml_dtypes.bfloat16)))
    return shards


_NC_CACHE = {}


def prep_w(W):
    """W [J,K,D,U] fp32 -> host-packed staging images (all bf16):
      wb  [128, J*G*16]  k-major  wb[khat*8+d, j, g, (h,uhat)]
      w0p [128, G*16]    wb[:, 0] / J  (uniform-softmax shortcut for j=0,t=0)
    where w_par[j,k,d,h,uhat] = W[j,k,d,2*uhat+h] (parity-split u)."""
    w = np.asarray(W, dtype=np.float32).reshape(J, K, D, 8, 2)
    w_par = w.transpose(0, 1, 2, 4, 3).reshape(J, G, 16, D, U)
    # wb: [khat, d, j, g, u]
    wb = w_par.transpose(2, 3, 0, 1, 4).reshape(128, J, G, U)
    w0p = wb[:, 0] * (1.0 / J)
    to = lambda a: np.ascontiguousarray(
        a.reshape(128, -1).astype(ml_dtypes.bfloat16))
    return to(wb), to(w0p)


def make_sel():
    """S[p, m] = 1 iff p == m//8: the selector matmul S^T @ ctmp broadcasts
    16-partition khat rows to all 128 k-major partitions."""
    S = np.zeros((128, 128), np.float32)
    for m in range(128):
        S[m // 8, m] = 1.0
    return np.ascontiguousarray(S.astype(ml_dtypes.bfloat16))


def kernel(inputs, W, num_outputs):
    assert int(num_outputs) == J
    x_full = np.asarray(inputs, dtype=np.float32)
    wb, w0p = prep_w(W)
    assert x_full.shape == (B, K, D, 1)

    if "nc" not in _NC_CACHE:
        _NC_CACHE["nc"] = build_nc()
    nc = _NC_CACHE["nc"]

    shards = shard_x(x_full)
    sel = make_sel()
    in_maps = [
        {"x_k": shards[i], "w_bands": wb, "w_j0p": w0p, "sel": sel}
        for i in range(N_CORES)
    ]
    res = bass_utils.run_bass_kernel_spmd(
        nc, in_maps, core_ids=list(range(N_CORES))
    )
    outs = [res.results[i]["out"] for i in range(N_CORES)]  # [b, J, U] each
    full = np.concatenate(outs, axis=0)  # [B, J, U]
    return full[..., None].astype(np.float32)


# revision 33
# speedup vs baseline: 1.0720x; 1.0196x over previous
"""CapsNet dynamic-routing kernel for Trainium2 (8 NeuronCores, SPMD).

Math (see reference):
  u_hat[j,b,k,u] = sum_d W[j,k,d,u] * x[b,k,d]
  for j in 0..9:  (sequential, b_IJ carried)
    3 routing iterations:
      c_k      = softmax(b_IJ, axis=1)[:, j]
      s[b,u]   = sum_k c_k u_hat[j,b,k,u]
      v        = squash(s)
      agree[k] = sum_{b,u} u_hat[j,b,k,u] v[b,u]   (sum over FULL batch)
      b_IJ[:, j] += agree
  out[b,j,u] = v (last iteration of each j)

Distribution: data-parallel over batch (64 per core).  The only cross-core
quantity is agree: per routing iteration each core ships its bf16 batch
partial [1152] through an AllGather (mesh floor ~4.6us vs AllReduce ~9.7us
at 8 cores) and a DVE tree sums the 8 partials locally in fp32.

Layout: single k-major partition order p = khat*8 + d for x, W, and all
routing state (v1 kept d-major duals of x and W just to make partition
broadcasts block-copyable).  The softmax state (e3, den) lives on the 16
khat partitions; c is broadcast to all 128 partitions by a selector matmul
c128 = S^T @ ctmp on the otherwise-idle PE (S[p,m] = 1 iff p == m//8, a
host input).  Stride-0-DMA partition broadcasts with a non-outermost
broadcast dim read garbage on HW, and engine copies cannot start at
non-32-aligned partitions, so the PE is the only clean path.

W band placement (block-diag wbd tiles for u_hat) is per-capsule 3-dim
scatter DMAs from the host-packed contiguous wb image, all routed through
the gpsimd SWDGE queue: its descriptor generator aggregates the 16-byte
runs into multi-partition packets.  v1 put 1/3 of these on the scalar
HWDGE queue, whose ~6k tiny packets per window starved the shared DMA
engines and delayed the collective return path by ~6us per iteration.

agree runs third-major into two bank-aligned PSUM slots (matmul
accumulation regions must not straddle the 2KB bank boundary); each third
drains to the khat-major bounce vector while the PE streams the next.

The PE duty-cycle governor (HAM, k=4/n=8 half-clock telemetry) tracks
recent PE activity density: junk fillers bridge the AllGather window so
the s/agree matmuls that follow are not half-clocked.  Oversized fillers
delay the softmax chain 1:1 (queue is in-order), so they are tuned small;
u_hat of the next capsule provides most of the keep-warm work.  fp8e4 for
the u_hat/agree path was tried and is NOT worth it: plain float8e4 does
not double-pump the PE (only the packed _x4 formats do) and the extra x/W
images cost startup time (measured 1074us vs 1052us, rel err 1.4e-2).

All matmul operands are bf16 (fp32 matmuls double-pass on the PE);
accumulation is fp32 in PSUM, and all routing state stays fp32.
(j=0,t=0) softmax is skipped: b_IJ=0 there so c=1/J exactly; the s matvec
uses a host-prescaled W_0/J tile, and runs before the u_hat(0) matmuls so
the (0,0) squash overlaps them.
Outputs accumulate in SBUF (v_all) and ship in one final DMA.

Measured on HW: 1052us (v1 AllReduce baseline: 1175us), rel err 3.1e-3.
"""

import numpy as np
import ml_dtypes

import concourse.bass as bass
import concourse.bacc as bacc
import concourse.mybir as mybir
import concourse.tile as tile
from concourse.tile import add_dep_helper
from concourse import bass_utils

F32 = mybir.dt.float32
BF16 = mybir.dt.bfloat16
AF = mybir.ActivationFunctionType
ALU = mybir.AluOpType

J = 10        # output capsules
K = 1152      # input capsules
D = 8         # in dim
U = 16        # out dim
B = 512       # batch
N_CORES = 8
ITERS = 3
EPS = 1e-7
G = K // 16   # 72 groups of 16 k

# wide junk fillers per AR window (t -> count); small: they only bridge the
# gap between the next capsule's u_hat work and the s matvec.
WIDE_FILL = {0: 10, 1: 10, 2: 10}
WIDE_FILL_LAST = 26   # j = J-1 windows have no next-capsule u_hat work
SQUASH_FILL = 0       # squash-gap fillers delayed agree 1:1 when half-clocked


def capsnet_body(tc, nc, x_dram, wb_dram, w0p_dram, sel_dram, out_dram,
                 replica_groups, b_local):
    """Emit the per-core program. x [128,G,b] bf16, wb [128, J*G*16] bf16,
    w0p [128, G*16] bf16, out [b, J, U] fp32."""
    from contextlib import ExitStack
    es = ExitStack()
    p_const = es.enter_context(tc.tile_pool(name="const", bufs=1))
    p_uhat = es.enter_context(tc.tile_pool(name="uhat", bufs=2))
    p_cw = es.enter_context(tc.tile_pool(name="cw", bufs=2))
    p_small = es.enter_context(tc.tile_pool(name="small", bufs=4))
    p_v = es.enter_context(tc.tile_pool(name="vpool", bufs=2))
    p_agr = es.enter_context(tc.tile_pool(name="agr", bufs=2))
    p_ps_uh = es.enter_context(tc.tile_pool(name="ps_uh", bufs=3, space="PSUM"))
    p_ps_s = es.enter_context(tc.tile_pool(name="ps_s", bufs=1, space="PSUM"))
    p_ps_a = es.enter_context(tc.tile_pool(name="ps_a", bufs=1, space="PSUM"))
    p_dram = es.enter_context(tc.tile_pool(name="dram", bufs=4, space="DRAM"))

    # ---- persistent tiles (all k-major: p = khat*8 + d)
    x_a = p_const.tile([128, G, b_local], BF16, tag="x_a")
    wbd_a = p_const.tile([128, G, 256], BF16, tag="wbd_a")
    wbd_b = p_const.tile([128, G, 256], BF16, tag="wbd_b")
    wbsrc = p_const.tile([128, J, G, U], BF16, tag="wbsrc")
    wj0p = p_const.tile([128, G, U], BF16, tag="wj0p")
    v_all = p_const.tile([b_local, J, U], F32, tag="v_all")
    # softmax state lives on the 16 khat partitions only; the final c is
    # broadcast to all 128 partitions by a selector matmul on the PE.
    e3 = p_const.tile([16, G, J], F32, tag="e3")
    den = p_const.tile([16, G], F32, tag="den")  # sum_j e3, kept incrementally
    sel = p_const.tile([128, 128], BF16, tag="sel")
    ctmp = p_const.tile([128, G], BF16, tag="ctmp")
    wbds = [wbd_a, wbd_b]

    # ---- startup loads: wbsrc capsules 0-1 first (gate the j0/j1 band
    # placements), x on gpsimd, rest of wbsrc behind.
    CH = G * U
    nc.scalar.dma_start(wj0p[:], w0p_dram.ap()[:])
    nc.scalar.dma_start(sel[:], sel_dram.ap()[:])
    wsv = wbsrc.rearrange("p j g u -> p (j g u)")
    nc.scalar.dma_start(wsv[:, 0:2 * CH], wb_dram.ap()[:, 0:2 * CH])
    for cidx in range(4):
        sl = slice(cidx * 18, (cidx + 1) * 18)
        nc.gpsimd.dma_start(x_a[:, sl, :], x_dram.ap()[:, sl])
    for j0 in range(2, J, 2):
        nc.scalar.dma_start(
            wsv[:, j0 * CH:(j0 + 2) * CH], wb_dram.ap()[:, j0 * CH:(j0 + 2) * CH]
        )

    # zeros for the block-diag tiles; bands only touch 1/16 of the cols so
    # the zero background is written once and never again.
    for h in range(2):
        nc.vector.memset(wbd_a[:, 36 * h:36 * (h + 1), :], 0.0)
    nc.vector.memset(e3[:], 1.0)
    nc.vector.memset(den[:], float(J))
    # rows 16..127 must stay 0 so the selector matmul contraction only
    # picks up the 16 live khat rows
    nc.vector.memset(ctmp[:], 0.0)

    A_tiles = {}

    def emit_band_place(j, rs=range(16)):
        """Place capsule j's block-diag W bands into wbds[j%2]: h-split
        3-dim DMAs from the host-packed contiguous wb_dram [128,(j g h u)].
        Engine tensor-copies can't do it (partition starts must be
        32-aligned; bands sit at 8-partition offsets).  All transfers ride
        the gpsimd SWDGE queue: its descriptor generator aggregates the
        16-byte runs into multi-partition packets, so the shared hardware
        DMA engines stay clean for the AllGather bounce path."""
        wbd = wbds[j % 2]
        dstv = wbd.rearrange("p g (h kk u) -> p g h kk u", h=2, kk=16)
        for r in rs:
            for h in range(2):
                src = bass.AP(
                    wb_dram, (8 * r) * (J * CH) + j * CH + h * 8,
                    [[J * CH, 8], [16, G], [1, 8]],
                )
                nc.gpsimd.dma_start(dstv[8 * r:8 * r + 8, :, h, r, :], src)

    def emit_uhat_mms(j, g_lo, g_hi, after=None):
        """PE matmuls + PSUM->SBUF copies for groups [g_lo, g_hi) of capsule j.
        `after`: ordering-only dep so the scheduler runs these in the
        AllGather window (after the agree matmuls), not earlier."""
        if j not in A_tiles:
            # partitions (h, b): even-u u_hat on 0..63, odd-u on 64..127
            A_tiles[j] = p_uhat.tile(
                [128, U // 2, G, 16], BF16, tag="uhat", name=f"uhat{j}"
            )
        A = A_tiles[j]
        wbd = wbds[j % 2]
        last_mm = None
        for gq in range(g_lo // 4, g_hi // 4):
            ps = p_ps_uh.tile([128, 512], F32, tag="ps_uh", name="ps_uh")
            for i in range(4):
                g = gq * 4 + i
                for h in range(2):
                    last_mm = nc.tensor.matmul(
                        ps[h * 64:(h + 1) * 64, i * 128:(i + 1) * 128],
                        x_a[:, g, :], wbd[:, g, h * 128:(h + 1) * 128],
                        start=True, stop=True,
                    )
                    if after is not None:
                        add_dep_helper(last_mm.ins, after.ins, sync=False,
                                       reason="uhat after agree")
            # all copies on DVE: a Copy activation on ACT would evict the
            # preloaded Sqrt/Exp tables and put a reload on the critical path.
            dst = A[:, :, gq * 4:gq * 4 + 4, :].transpose((0, 2, 1, 3))
            src_v = ps.rearrange("p (g k u) -> p g k u", k=16, u=U // 2)
            src_v = src_v.transpose((0, 1, 3, 2))
            nc.vector.tensor_copy(dst, src_v)
        return last_mm

    p_ps_f = es.enter_context(tc.tile_pool(name="ps_f", bufs=1, space="PSUM"))

    for _f in range(30):
        fwarm = p_ps_f.tile([b_local, 512], F32, tag="fps", name="fwarm")
        nc.tensor.matmul(fwarm[:, 0:128], sel[:, 0:b_local], sel[:],
                         start=True, stop=True)

    def emit_filler(j, n, after=None):
        """Independent wide matmuls with no consumers: keep the PE warm
        while real work is blocked (results are discarded).  N=512.  Reads
        x_a/wbsrc only -- touching wbd here would make the band-placement
        copies wait on filler drains."""
        wf = wbsrc.rearrange("p j g u -> p (j g u)")
        fps = p_ps_f.tile([b_local, 512], F32, tag="fps", name="fps")
        first_mm = last_mm = None
        for f in range(n):
            fs = f % 22
            last_mm = nc.tensor.matmul(
                fps[:], x_a[:, f % G, :], wf[:, fs * 512:(fs + 1) * 512],
                start=True, stop=True,
            )
            if first_mm is None:
                first_mm = last_mm
            if after is not None:
                add_dep_helper(last_mm.ins, after.ins, sync=False,
                               reason="filler ordering")
        return first_mm, last_mm

    pe_tail = None  # ordering anchor: last PE inst of the prev AG window

    for j in range(J):
        A = None
        for t in range(ITERS):
            last = (j == J - 1) and (t == ITERS - 1)
            first = (j == 0) and (t == 0)
            if first:
                # b_IJ = 0 => c = 1/J exactly; use the host-prescaled W_0/J
                cw = wj0p
            else:
                # softmax column j: c = e3[:,:,j] / den (den kept incrementally)
                rec = p_small.tile([16, G], F32, tag="rec")
                nc.vector.reciprocal(rec[:], den[:])
                # ctmp rows 16..127 are zeroed once at startup; the selector
                # matmul c128[m, g] = sum_p S[p, m] ctmp[p, g] = ctmp[m//8, g]
                # broadcasts c to all 128 k-major partitions on the (idle)
                # PE.  Stride-0 DMA partition broadcasts with a non-outer
                # broadcast dim read garbage on HW, hence this route.
                nc.vector.tensor_mul(ctmp[0:16, :], e3[:, :, j], rec[:])
                c_ps = p_ps_s.tile([128, G], F32, tag="c_ps", name="c_ps")
                cmm = nc.tensor.matmul(c_ps[:], sel[:], ctmp[:],
                                       start=True, stop=True)
                if pe_tail is not None:
                    add_dep_helper(cmm.ins, pe_tail.ins, sync=False,
                                   reason="c bcast after AG-window fillers")
                # cW = W_j * c (c broadcast over u, read from PSUM); two
                # halves so the s matvec can start while the second half is
                # still computing
                cw = p_cw.tile([128, G, U], BF16, tag="cw")
                GH = G // 2
                for h in range(2):
                    sl = slice(h * GH, (h + 1) * GH)
                    nc.vector.tensor_mul(
                        cw[:, sl, :], wbsrc[:, j, sl, :],
                        c_ps[:, sl].unsqueeze(2).broadcast_to((128, GH, U)),
                    )
            # s matvec: accumulate over groups
            s_ps = p_ps_s.tile([b_local, U], F32, tag="s_ps")
            for g in range(G):
                mm = nc.tensor.matmul(
                    s_ps[:], x_a[:, g, :], cw[:, g, :],
                    start=(g == 0), stop=(g == G - 1),
                )
                if g == 0 and pe_tail is not None:
                    add_dep_helper(mm.ins, pe_tail.ins, sync=False,
                                   reason="s after AG-window fillers")
            if not last:
                emit_filler(j, SQUASH_FILL, after=mm)
            # squash: v = s * ssq / ((1+ssq)(sqrt(ssq)+EPS))
            # ssq via DVE mult+reduce (keeps ACT on the Sqrt table)
            s_sb = p_small.tile([b_local, U], F32, tag="s_sb")
            shadow = p_small.tile([b_local, U], F32, tag="shadow")
            ssq = p_small.tile([b_local, 1], F32, tag="ssq")
            sq1 = p_small.tile([b_local, 1], F32, tag="sq1")
            sqr = p_small.tile([b_local, 1], F32, tag="sqr")
            dn2 = p_small.tile([b_local, 1], F32, tag="dn2")
            rc2 = p_small.tile([b_local, 1], F32, tag="rc2")
            fac = p_small.tile([b_local, 1], F32, tag="fac")
            nc.vector.tensor_copy(s_sb[:], s_ps[:])
            nc.vector.tensor_mul(shadow[:], s_sb[:], s_sb[:])
            nc.vector.tensor_reduce(ssq[:], shadow[:], mybir.AxisListType.X, ALU.add)
            nc.scalar.sqrt(sqr[:], ssq[:])
            nc.vector.tensor_scalar_add(sq1[:], ssq[:], 1.0)
            nc.vector.scalar_tensor_tensor(
                dn2[:], sqr[:], EPS, sq1[:], ALU.add, ALU.mult
            )
            nc.vector.reciprocal(rc2[:], dn2[:])
            nc.vector.tensor_mul(fac[:], ssq[:], rc2[:])
            if not last:
                # preload the Exp ACT table during the AllGather window
                # (anchored on fac so it runs after this squash)
                dxp = p_small.tile([b_local, 1], F32, tag="dxp")
                nc.scalar.activation(dxp[:], fac[:], AF.Exp)
            if first:
                # u_hat(0) band placement + matmuls run on the PE while the
                # (0,0) squash proceeds on DVE: the s matvec above only
                # needed x_a + the prescaled W_0/J tile
                emit_band_place(0)
                emit_uhat_mms(0, 0, G)
                for h2 in range(2):
                    nc.vector.memset(wbd_b[:, 36 * h2:36 * (h2 + 1), :], 0.0)
                emit_band_place(1)
            if t == ITERS - 1:
                # v cols are parity-ordered (h,uhat); un-permute into the
                # SBUF output accumulator (single strided DVE op)
                dstv = v_all[:, j, :].rearrange("b (u h) -> b u h", h=2)
                srcv = s_ps.rearrange("b (h u) -> b h u", h=2)
                srcv = srcv.transpose((0, 2, 1))
                nc.vector.tensor_scalar_mul(dstv, srcv, fac[:])
            if last:
                pe_tail = None
                break
            # agree matvec, third-major: each PSUM third is drained to the
            # k-major bounce vector while the PE streams the next third.
            if A is None:
                A = A_tiles.pop(j)
            v_bf = p_v.tile([b_local, U], BF16, tag="v_bf")
            nc.vector.tensor_scalar_mul(v_bf[:], s_ps[:], fac[:])
            # v2[(h,b), q] = v[b, 2q+h]: contract over 128 partitions.
            v2 = p_v.tile([128, U // 2], BF16, tag="v2")
            nc.vector.tensor_copy(v2[0:64, :], v_bf[:, 0:8])
            nc.vector.tensor_copy(v2[64:128, :], v_bf[:, 8:16])
            # two 384-col slots (1 PSUM bank each); third 2 reuses slot 0
            # after its drain, which completes during third 1's compute
            aps3 = p_ps_a.tile([1, 1024], F32, tag="ps_a3", name="ps_a3")
            agr_sb = p_agr.tile([1, K], BF16, tag="agr_sb")
            sb_v = agr_sb.rearrange("p (k c g) -> p k c g", k=16, c=3)
            cc_in = p_dram.tile([1, K], BF16, tag="cc_in")
            ci_v = cc_in.rearrange("p (k c g) -> p k c g", k=16, c=3)
            agree_last = None
            for third in range(3):
                off = (third % 2) * 512
                for q in range(U // 2):
                    agree_last = nc.tensor.matmul(
                        aps3[:, off:off + 384],
                        v2[:, q:q + 1],
                        A[:, q, third * 24:(third + 1) * 24, :],
                        start=(q == 0), stop=(q == U // 2 - 1),
                    )
                src_v = aps3[:, off:off + 384]
                src_v = src_v.rearrange("p (g k) -> p k g", k=16)
                nc.vector.tensor_copy(sb_v[:, :, third, :], src_v)
                if third == 1:
                    # ship thirds 0-1 while the PE streams third 2
                    nc.sync.dma_start(ci_v[:, :, 0:2, :], sb_v[:, :, 0:2, :])
            fill_j = j + 1 if j + 1 < J else j
            # collective bounce: agr_sb is khat-major so the replicated
            # return DMA has 288-byte contiguous runs.
            cc_out = nc.dram_tensor(
                f"ccout_{j}_{t}", [N_CORES, K], BF16, addr_space="Shared"
            )
            nc.sync.dma_start(ci_v[:, :, 2, :], sb_v[:, :, 2, :])
            cc_inst = nc.gpsimd.collective_compute(
                "AllGather", ALU.bypass,
                replica_groups=replica_groups,
                ins=[cc_in[:].opt()], outs=[cc_out.ap().opt()],
            )
            # PE schedule for the AG window: a short tail filler covers the
            # bounce copy/DMA, then the next capsule's u_hat, then a few
            # junk fillers; the PE then blocks at the next s matvec
            # (ordering dep via pe_tail).
            _, tail = emit_filler(fill_j, 6, after=agree_last)
            if j + 1 < J:
                tail = emit_uhat_mms(j + 1, t * 24, (t + 1) * 24, after=tail)
                wide_n = WIDE_FILL[t]
            else:
                wide_n = WIDE_FILL_LAST
            _, pe_tail = emit_filler(fill_j, wide_n, after=tail)
            # AllGather return: one 3-dim DMA onto the 16 khat partitions
            # [16, rank, G]; a 3-op DVE tree reduces the 8 rank partials.
            agr8 = p_agr.tile([16, N_CORES, G], BF16, tag="agr8")
            t8 = p_agr.tile([16, 4, G], F32, tag="t8")
            t4 = p_agr.tile([16, 2, G], F32, tag="t4")
            agr = p_agr.tile([16, G], F32, tag="agr")
            eag = p_agr.tile([16, G], F32, tag="eag")
            # four queues quarter the return transfer's serial latency; the
            # gpsimd quarter is its first instruction after the AG wait
            for qi, eng in enumerate((nc.sync, nc.scalar, nc.sync,
                                      nc.gpsimd)):
                src_q = bass.AP(cc_out, qi * 2 * K, [[G, 16], [K, 2], [1, G]])
                eng.dma_start(agr8[:, qi * 2:(qi + 1) * 2, :], src_q)
            # rank-partials reduce on vector: 3-op tree, fp32 accumulation
            # from the bf16 wire payloads
            nc.vector.tensor_add(t8[:], agr8[:, 0:4, :], agr8[:, 4:8, :])
            nc.vector.tensor_add(t4[:], t8[:, 0:2, :], t8[:, 2:4, :])
            nc.vector.tensor_add(agr[:], t4[:, 0, :], t4[:, 1, :])
            if j + 2 < J:
                # place j+2's W bands into wbds[j%2] (last reader u_hat(j)
                # finished during capsule j-1, so these never block); the
                # transfers process in the post-AG chain's shadow
                rs = (range(0, 6), range(6, 11), range(11, 16))[t]
                emit_band_place(j + 2, rs)
            nc.scalar.activation(eag[:], agr[:], AF.Exp)
            # preload Sqrt table for the next squash (anchored on eag)
            dsq = p_small.tile([16, 1], F32, tag="dsq")
            nc.scalar.activation(dsq[:], eag[0:16, 0:1], AF.Sqrt)
            # delta = (eag-1)*e3_j keeps den incremental; then update e3
            delta = p_small.tile([16, G], F32, tag="delta")
            nc.vector.scalar_tensor_tensor(
                delta[:], eag[:], -1.0, e3[:, :, j], ALU.add, ALU.mult
            )
            nc.vector.tensor_mul(e3[:, :, j], e3[:, :, j], eag[:])
            nc.vector.tensor_add(den[:], den[:], delta[:])

    # single output DMA at the end
    nc.sync.dma_start(out_dram.ap()[:], v_all[:])
    es.close()


def build_nc(n_cores=N_CORES, b_local=B // N_CORES):
    nc = bacc.Bacc(
        "TRN2", target_bir_lowering=False, debug=False,
        num_devices=n_cores,
    )
    x_dram = nc.dram_tensor("x_k", [128, G, b_local], BF16, kind="ExternalInput")
    wb_dram = nc.dram_tensor("w_bands", [128, J * G * U], BF16, kind="ExternalInput")
    w0p_dram = nc.dram_tensor("w_j0p", [128, G * U], BF16, kind="ExternalInput")
    sel_dram = nc.dram_tensor("sel", [128, 128], BF16, kind="ExternalInput")
    out_dram = nc.dram_tensor("out", [b_local, J, U], F32, kind="ExternalOutput")
    rg = [list(range(n_cores))]
    with tile.TileContext(nc) as tc:
        capsnet_body(tc, nc, x_dram, wb_dram, w0p_dram, sel_dram, out_dram,
                     rg, b_local)
    nc.compile()
    return nc


def shard_x(x_full):
    """x_full [B,1152,8,1] -> per-core [128, G, b] bf16 k-major
    (p = khat*8 + d)."""
    b_local = x_full.shape[0] // N_CORES
    shards = []
    for i in range(N_CORES):
        xs = np.ascontiguousarray(
            x_full[i * b_local:(i + 1) * b_local, :, :, 0], dtype=np.float32
        )
        r = xs.reshape(b_local, G, 16, D)
        x_a = r.transpose(2, 3, 1, 0).reshape(128, G, b_local)  # k-major
        shards.append(np.ascontiguousarray(x_a.astype(ml_dtypes.bfloat16)))
    return shards


_NC_CACHE = {}


def prep_w(W):
    """W [J,K,D,U] fp32 -> host-packed staging images (all bf16):
      wb  [128, J*G*16]  k-major  wb[khat*8+d, j, g, (h,uhat)]
      w0p [128, G*16]    wb[:, 0] / J  (uniform-softmax shortcut for j=0,t=0)
    where w_par[j,k,d,h,uhat] = W[j,k,d,2*uhat+h] (parity-split u)."""
    w = np.asarray(W, dtype=np.float32).reshape(J, K, D, 8, 2)
    w_par = w.transpose(0, 1, 2, 4, 3).reshape(J, G, 16, D, U)
    # wb: [khat, d, j, g, u]
    wb = w_par.transpose(2, 3, 0, 1, 4).reshape(128, J, G, U)
    w0p = wb[:, 0] * (1.0 / J)
    to = lambda a: np.ascontiguousarray(
        a.reshape(128, -1).astype(ml_dtypes.bfloat16))
    return to(wb), to(w0p)


def make_sel():
    """S[p, m] = 1 iff p == m//8: the selector matmul S^T @ ctmp broadcasts
    16-partition khat rows to all 128 k-major partitions."""
    S = np.zeros((128, 128), np.float32)
    for m in range(128):
        S[m // 8, m] = 1.0
    return np.ascontiguousarray(S.astype(ml_dtypes.bfloat16))


def kernel(inputs, W, num_outputs):
    assert int(num_outputs) == J
    x_full = np.asarray(inputs, dtype=np.float32)
    wb, w0p = prep_w(W)
    assert x_full.shape == (B, K, D, 1)

    if "nc" not in _NC_CACHE:
        _NC_CACHE["nc"] = build_nc()
    nc = _NC_CACHE["nc"]

    shards = shard_x(x_full)
    sel = make_sel()
    in_maps = [
        {"x_k": shards[i], "w_bands": wb, "w_j0p": w0p, "sel": sel}
        for i in range(N_CORES)
    ]
    res = bass_utils.run_bass_kernel_spmd(
        nc, in_maps, core_ids=list(range(N_CORES))
    )
    outs = [res.results[i]["out"] for i in range(N_CORES)]  # [b, J, U] each
    full = np.concatenate(outs, axis=0)  # [B, J, U]
    return full[..., None].astype(np.float32)
